# revision 5
# baseline (speedup 1.0000x reference)
"""Causal self-attention with RoPE on 8 Trainium2 NeuronCores (v5).

Problem: B=2, S=2048, H=16 heads, D=128, HID=2048, fp32.
  qkv = x @ w_qkv.T ; RoPE(q, k) ; causal softmax(q k^T / sqrt(D)) @ v ; out @ w_o.T

Sharding (hardcoded): core c handles batch b = c // 4 and head group
g = c % 4 (heads 4g..4g+4). Each core computes a partial (S, HID) output
contracted over its 512 hidden dims of the o-projection; the host sums the 4
partials per batch.

All matmuls run in fp32r (TF32-class): at moving dims >=256 fp32r streams
1 column/cycle at full clock; bf16 draws more PE power and downclocks
(~259 ns vs ~227 ns per 512-col matmul, measured).

Engine discipline (engine queues are strict in-order FIFOs, so latency
coupling matters more than busy%):
 - ACT: all PSUM->SBUF copies + exp/ln. DVE: RoPE + mask adds + normalize
   muls only, so a copy never queues behind a stalled RoPE/normalize.
 - The softmax reciprocal is split in three (ln+exp on ACT / broadcast on
   GpSimd / multiply on DVE) and each stage is emitted only at a program
   point where its inputs are already done, so no FIFO head-blocks.
 - Phase C is NOT interleaved into B: C units gate on normalize chains and
   would stall the in-order PE stream (measured regression).

DMA discipline: per-dma_start completion bandwidth is only ~40 GB/s
(aggregate across queues ~330 GB/s), so the first x chunks are split into
column-halves across two rings, weights are host-prepped contiguous, and
issue order matches consumption order. The v projection runs as a single
kc pass so each x chunk's SBUF slot retires early — the next half's x
refills stream in under the whole v pass and the half boundary has no
DMA bubble.
"""

import os

import numpy as np

import concourse.bacc as bacc
import concourse.tile as tile
from concourse import mybir
from concourse.bass_utils import run_bass_kernel_spmd

B, S, H, D = 2, 2048, 16, 128
HID = H * D
THETA = 10000.0
SCALE = 1.0 / float(np.sqrt(D))
NH = 4                 # heads per core
NC = 8                 # cores
NKC = HID // 128       # contraction chunks (128 wide)
SB = 512               # attention si-block / moving dim
NSB = S // SB          # si blocks
SH = S // 2            # s-half
F32 = mybir.dt.float32

MM_MODE = os.environ.get("BASS_MM_MODE", "fp32r")
MMDT = mybir.dt.float32r if MM_MODE == "fp32r" else mybir.dt.float32

LAST_RESULT = None  # BassKernelResults of the most recent run (for test harness)


def _build_nc():
    nc = bacc.Bacc("TRN2", target_bir_lowering=False, debug=False, num_devices=NC)

    xT = nc.dram_tensor("xT", [HID, S], F32, kind="ExternalInput")
    wqk = nc.dram_tensor("wqk", [2 * NH, 128, HID], F32, kind="ExternalInput")
    wv = nc.dram_tensor("wv", [NKC, 128, NH * 128], F32, kind="ExternalInput")
    woT = nc.dram_tensor("woT", [NH * 128, HID], F32, kind="ExternalInput")
    cosT = nc.dram_tensor("cosT", [D, S], F32, kind="ExternalInput")
    sinST = nc.dram_tensor("sinST", [D, S], F32, kind="ExternalInput")
    maskadd = nc.dram_tensor("maskadd", [128, 128], F32, kind="ExternalInput")
    out = nc.dram_tensor("out", [S, HID], F32, kind="ExternalOutput")

    with tile.TileContext(nc) as tc:
        with tc.tile_pool(name="pconst", bufs=1) as pconst, \
             tc.tile_pool(name="pqk", bufs=1) as pqk, \
             tc.tile_pool(name="pvn", bufs=1) as pvn:

            ones_f = pconst.tile([128, 4], F32, name="ones_f")
            nc.vector.memset(ones_f, 1.0)
            ones4 = pconst.tile([128, 4], MMDT, name="ones4")
            nc.vector.tensor_copy(ones4, ones_f)
            tri_t = pconst.tile([128, 128], F32, name="tri")
            nc.scalar.dma_start(out=tri_t, in_=maskadd[:, :])

            qT = [pqk.tile([128, S], MMDT, name=f"qT_{h}") for h in range(NH)]
            kT = [pqk.tile([128, S], MMDT, name=f"kT_{h}") for h in range(NH)]
            vn = [pvn.tile([128, 4, NH * 128], MMDT, name=f"vn_{g}")
                  for g in range(4)]
            outT = qT  # attention output aliases qT per si-block

            # ---- Phase A: q/k/v projection + RoPE, per s-half ----
            with tc.tile_pool(name="px", bufs=1) as px, \
                 tc.tile_pool(name="pwq", bufs=3) as pwq, \
                 tc.tile_pool(name="pwv", bufs=4) as pwvp, \
                 tc.tile_pool(name="ptrig", bufs=1) as ptrig, \
                 tc.tile_pool(name="psh", bufs=1) as psh:
                for half in range(2):
                    s0 = half * SH
                    xh = []
                    for kc in range(NKC):
                        xt = px.tile([128, SH], MMDT, name=f"xh{kc}")
                        src = xT[kc * 128:(kc + 1) * 128, s0:s0 + SH]
                        if half == 0 and kc < 4:
                            # single-DMA completion BW is ~40 GB/s: split the
                            # startup-critical chunks across two rings
                            nc.sync.dma_start(
                                out=xt[:, 0:SB],
                                in_=src[:, 0:SB].bitcast(MMDT))
                            nc.gpsimd.dma_start(
                                out=xt[:, SB:SH],
                                in_=src[:, SB:SH].bitcast(MMDT))
                        else:
                            eng = nc.sync if kc % 2 == 0 else nc.gpsimd
                            eng.dma_start(out=xt, in_=src.bitcast(MMDT))
                        xh.append(xt)
                    cos_t = ptrig.tile([D, SH], F32, name="cosT")
                    sin_t = ptrig.tile([D, SH], F32, name="sinST")
                    nc.scalar.dma_start(out=cos_t, in_=cosT[:, s0:s0 + SH])
                    nc.scalar.dma_start(out=sin_t, in_=sinST[:, s0:s0 + SH])

                    # q/k projection, transposed output [d, s], then RoPE.
                    # All PSUM->SBUF copies on ACT: the DVE FIFO holds only
                    # RoPE ops, so a late cos/sin or shuffle never delays
                    # PSUM bank recycling.
                    ppa_cm = tc.tile_pool(name="ppa", bufs=8, space="PSUM")
                    ppa = ppa_cm.__enter__()
                    for h in range(NH):
                        for kind, dst in ((0, qT[h]), (1, kT[h])):
                            ot = kind * NH + h
                            wt = pwq.tile([128, HID], MMDT, name="wqk")
                            for pc in range(2):
                                nc.scalar.dma_start(
                                    out=wt[:, pc * 1024:(pc + 1) * 1024],
                                    in_=wqk[ot][:, pc * 1024:(pc + 1) * 1024]
                                    .bitcast(MMDT))
                            ps0 = ppa.tile([128, SB], F32, name="qkps")
                            ps1 = ppa.tile([128, SB], F32, name="qkps")
                            for kc in range(NKC):
                                nc.tensor.matmul(
                                    ps0, wt[:, kc * 128:(kc + 1) * 128],
                                    xh[kc][:, 0:SB],
                                    start=(kc == 0), stop=(kc == NKC - 1))
                                nc.tensor.matmul(
                                    ps1, wt[:, kc * 128:(kc + 1) * 128],
                                    xh[kc][:, SB:SH],
                                    start=(kc == 0), stop=(kc == NKC - 1))
                            nc.scalar.copy(out=dst[:, s0:s0 + SB], in_=ps0)
                            nc.scalar.copy(out=dst[:, s0 + SB:s0 + SH], in_=ps1)
                            # RoPE in place (rotate-half partition swap by DMA)
                            sl = dst[:, s0:s0 + SH]
                            sh_t = psh.tile([128, SH], MMDT, name="shuf")
                            nc.gpsimd.dma_start(out=sh_t[0:64, :],
                                                in_=dst[64:128, s0:s0 + SH])
                            nc.gpsimd.dma_start(out=sh_t[64:128, :],
                                                in_=dst[0:64, s0:s0 + SH])
                            nc.vector.tensor_mul(sh_t, sh_t, sin_t)
                            nc.vector.tensor_mul(sl, sl, cos_t)
                            nc.vector.tensor_add(sl, sl, sh_t)
                    ppa_cm.__exit__(None, None, None)

                    # v projection, natural layout [s, 4 heads x d]: one kc
                    # pass over 8 PSUM banks, so each x chunk's slot retires
                    # at its kc step and the next half's refill overlaps the
                    # whole pass
                    pvp_cm = tc.tile_pool(name="pvp", bufs=8, space="PSUM")
                    pvp = pvp_cm.__enter__()
                    wv_t = []
                    for kc in range(NKC):
                        wvt = pwvp.tile([128, NH * 128], MMDT, name="wv")
                        nc.scalar.dma_start(out=wvt, in_=wv[kc].bitcast(MMDT))
                        wv_t.append(wvt)
                    vps = [pvp.tile([128, NH * 128], F32, name="vps")
                           for _ in range(8)]
                    for kc in range(NKC):
                        for st in range(8):
                            nc.tensor.matmul(
                                vps[st], xh[kc][:, st * 128:(st + 1) * 128],
                                wv_t[kc],
                                start=(kc == 0), stop=(kc == NKC - 1))
                    for st in range(8):
                        sg = half * 8 + st   # global s-chunk
                        nc.scalar.copy(out=vn[sg // 4][:, sg % 4, :],
                                       in_=vps[st])
                    pvp_cm.__exit__(None, None, None)

            # o-proj weights: issued at B start, resident long before C
            pwo_cm = tc.tile_pool(name="pwo", bufs=1)
            pwo = pwo_cm.__enter__()
            wo_t = []
            for h in range(NH):
                wot = pwo.tile([128, HID], MMDT, name=f"wo{h}")
                nc.scalar.dma_start(
                    out=wot,
                    in_=woT[h * 128:(h + 1) * 128, :].bitcast(MMDT))
                wo_t.append(wot)

            # ---- Phase B: attention per (si-block round, head) ----
            # Softmax denominator: e-tiles are accumulated on DVE/GpSimd into
            # e_acc (instead of streaming every chunk through a [128,4]-ones
            # matmul -> removes ~70K columns from the tensor stream); one
            # 512-col ones-matmul per (h, sib) reduces e_acc partitions, then
            # DVE reciprocal_approx_fast (18 bits) replaces the ACT ln/exp
            # pair. The 4 stages (ones-mm -> rec -> broadcast -> normalize
            # mul) advance one step per chunk-pair slot so every op reaches
            # its in-order FIFO only when its inputs are already done.
            pending = []   # [stage, h, si0, o_ps, e_acc, l4, rb]

            with tc.tile_pool(name="pexp", bufs=4) as pexp, \
                 tc.tile_pool(name="pea", bufs=2) as pea, \
                 tc.tile_pool(name="prr", bufs=2) as prr, \
                 tc.tile_pool(name="prb", bufs=2) as prb, \
                 tc.tile_pool(name="psc", bufs=2, space="PSUM") as psc, \
                 tc.tile_pool(name="plp", bufs=1, space="PSUM") as plp, \
                 tc.tile_pool(name="pop", bufs=3, space="PSUM") as pop:

                def advance(item):
                    st = item[0]
                    _, h, si0, o_ps, e_acc, l4, rb = item
                    if st == 0:
                        l4 = plp.tile([4, SB], F32, name="l4")
                        nc.tensor.matmul(l4, ones4, e_acc,
                                         start=True, stop=True)
                        item[5] = l4
                    elif st == 1:
                        rec = prr.tile([1, SB], F32, name="rec")
                        nc.vector.reciprocal_approx_fast(
                            out=rec, in_=l4[0:1, :])
                        rb = prb.tile([128, SB], F32, name="rb")
                        nc.gpsimd.partition_broadcast(rb, rec)
                        item[6] = rb
                    else:
                        nc.vector.tensor_mul(outT[h][:, si0:si0 + SB],
                                             o_ps, rb)
                    item[0] += 1
                    return item[0] >= 3

                def slot(budget):
                    done = 0
                    for item in list(pending):
                        if done >= budget:
                            break
                        if advance(item):
                            pending.remove(item)
                        done += 1

                for sib in range(NSB):
                    si0 = sib * SB
                    nch = 4 * (sib + 1)
                    for h in range(NH):
                        o_ps = pop.tile([128, SB], F32, name="ops")
                        e_acc = pea.tile([128, SB], MMDT, name="eacc")
                        for cp in range(nch // 2):
                            s_ps = psc.tile([128, 2, SB], F32, name="sps")
                            e_t = pexp.tile([128, 2, SB], MMDT, name="exp")
                            los = []
                            for j in range(2):
                                cj = cp * 2 + j
                                dg = cj - (nch - 4)
                                lo = dg * 128 if dg > 0 else 0
                                los.append((cj, lo))
                                nc.tensor.matmul(
                                    s_ps[:, j, lo:],
                                    kT[h][:, cj * 128:(cj + 1) * 128],
                                    qT[h][:, si0 + lo:si0 + SB],
                                    start=True, stop=True)
                                if dg >= 0:
                                    nc.vector.tensor_add(
                                        s_ps[:, j, lo:lo + 128],
                                        s_ps[:, j, lo:lo + 128], tri_t)
                            # exp only over the valid [lo:] ranges (the stale
                            # sub-diagonal region was ~15% wasted ACT time)
                            if los[0][1] == 0 and los[1][1] == 0:
                                nc.scalar.activation(
                                    out=e_t, in_=s_ps,
                                    func=mybir.ActivationFunctionType.Exp,
                                    scale=SCALE)
                            else:
                                for j in range(2):
                                    lo = los[j][1]
                                    nc.scalar.activation(
                                        out=e_t[:, j, lo:],
                                        in_=s_ps[:, j, lo:],
                                        func=mybir.ActivationFunctionType.Exp,
                                        scale=SCALE)
                            # previous heads' deferred normalize stages
                            slot(2 if sib == 0 else 1)
                            # accumulate e into e_acc (DVE + GpSimd split so
                            # neither FIFO carries the whole load)
                            lo0, lo1 = los[0][1], los[1][1]
                            if cp == 0:
                                if lo1 == 0:
                                    nc.vector.tensor_add(
                                        e_acc, e_t[:, 0, :], e_t[:, 1, :])
                                else:
                                    nc.vector.tensor_copy(e_acc, e_t[:, 0, :])
                                    nc.gpsimd.tensor_add(
                                        e_acc[:, lo1:], e_acc[:, lo1:],
                                        e_t[:, 1, lo1:])
                            else:
                                nc.vector.tensor_add(
                                    e_acc[:, lo0:], e_acc[:, lo0:],
                                    e_t[:, 0, lo0:])
                                nc.gpsimd.tensor_add(
                                    e_acc[:, lo1:], e_acc[:, lo1:],
                                    e_t[:, 1, lo1:])
                            for j in range(2):
                                cj, lo = los[j]
                                nc.tensor.matmul(
                                    o_ps[:, lo:],
                                    vn[cj // 4][:, cj % 4,
                                                h * 128:(h + 1) * 128],
                                    e_t[:, j, lo:],
                                    start=(cj == 0), stop=(cj == nch - 1))
                        pending.append([0, h, si0, o_ps, e_acc, None, None])
                    # round end: drain enough that at most one item (the
                    # just-finished head) stays in flight into the next round
                    while len(pending) > 1:
                        if advance(pending[0]):
                            pending.pop(0)
                while pending:
                    if advance(pending[0]):
                        pending.pop(0)

            # ---- Phase C: partial o-projection ----
            with tc.tile_pool(name="pft", bufs=4, space="PSUM") as pft, \
                 tc.tile_pool(name="pst", bufs=4) as pst:
                u = 0
                for st in range(S // 128):
                    for ob in range(HID // SB):
                        fin = pft.tile([128, SB], F32, name="fin")
                        for hh in range(NH):
                            nc.tensor.matmul(
                                fin, outT[hh][:, st * 128:(st + 1) * 128],
                                wo_t[hh][:, ob * SB:(ob + 1) * SB],
                                start=(hh == 0), stop=(hh == NH - 1))
                        stg = pst.tile([128, SB], F32, name="stg")
                        if u % 2 == 0:
                            nc.scalar.copy(out=stg, in_=fin)
                        else:
                            nc.vector.tensor_copy(stg, fin)
                        # alternate output rings so the final drain halves
                        eng = nc.sync if u % 2 == 0 else nc.gpsimd
                        u += 1
                        eng.dma_start(
                            out=out[st * 128:(st + 1) * 128,
                                    ob * SB:(ob + 1) * SB],
                            in_=stg)

            pwo_cm.__exit__(None, None, None)

    # Force exp and ln onto the single `natural_log_exp_and_others` ACT
    # table set: with the default map the table-load pass alternates between
    # the exp-only and ln-only sets (~2.7us per reload on ScalarE). Blank
    # the single-function sets (positions preserved, so set ids stay valid)
    # so both functions resolve to the combined set -> one load.
    import concourse.bacc as _bacc_mod
    import concourse.hw_specs as _hw_specs
    _orig_tables = _hw_specs.get_activation_tables

    def _patched_tables(arch):
        t = dict(_orig_tables(arch))
        for name in ("exp_and_others", "exp_and_friends", "natural_log"):
            if name in t:
                t[name] = set()
        return t

    _bacc_mod.get_activation_tables = _patched_tables
    try:
        nc.compile()
    finally:
        _bacc_mod.get_activation_tables = _orig_tables
    return nc


_NC_CACHE = None


def _get_nc():
    global _NC_CACHE
    if _NC_CACHE is None:
        _NC_CACHE = _build_nc()
    return _NC_CACHE


def _host_inputs(x, w_qkv, w_o):
    """Per-core input maps (sharding + contiguous-DMA layout prep on host)."""
    inv_freq = 1.0 / (THETA ** (np.arange(0, D, 2, dtype=np.float64) / D))
    pos = np.arange(S, dtype=np.float64)
    freqs = pos[:, None] * inv_freq[None, :]          # (S, D/2)
    emb = np.concatenate([freqs, freqs], axis=-1)     # (S, D)
    cosT = np.ascontiguousarray(np.cos(emb).T.astype(np.float32))   # (D, S)
    sign = np.concatenate([-np.ones(D // 2), np.ones(D // 2)])
    sinST = np.ascontiguousarray((sign[None, :] * np.sin(emb)).T
                                 .astype(np.float32))               # (D, S)
    # additive causal triangle for a diagonal 128x128 block of scores^T:
    # keep (add 0) when sj_local <= si_local, else -1e30
    p = np.arange(128)[:, None]
    f = np.arange(128)[None, :]
    maskadd = np.where(p <= f, 0.0, -1e30).astype(np.float32)       # (128, 128)

    xTb = [np.ascontiguousarray(x[b].T) for b in range(B)]          # (HID, S)
    in_maps = []
    for c in range(NC):
        b, g = c // 4, c % 4
        rows = slice(g * NH * D, (g + 1) * NH * D)
        wq = w_qkv[0 * HID:1 * HID][rows]             # (512, 2048)
        wk = w_qkv[1 * HID:2 * HID][rows]
        wvm = w_qkv[2 * HID:3 * HID][rows]
        # wqk[ot][p, kc*128+od] = w[ot*128+od, kc*128+p]
        wqk_arr = np.empty((2 * NH, 128, HID), dtype=np.float32)
        for kind, wm in ((0, wq), (1, wk)):
            for h in range(NH):
                wT = wm[h * 128:(h + 1) * 128].T      # (2048 hid, 128 od)
                wqk_arr[kind * NH + h] = (
                    wT.reshape(NKC, 128, 128).transpose(1, 0, 2)
                    .reshape(128, HID))
        # wv[kc][p, j] = w_v[j, kc*128+p]
        wv_arr = np.ascontiguousarray(
            wvm.T.reshape(NKC, 128, NH * 128)).astype(np.float32)
        woT = np.ascontiguousarray(w_o[:, rows].T).astype(np.float32)
        in_maps.append({
            "xT": xTb[b], "wqk": wqk_arr, "wv": wv_arr, "woT": woT,
            "cosT": cosT, "sinST": sinST, "maskadd": maskadd,
        })
    return in_maps


def kernel(x, w_qkv, w_o):
    global LAST_RESULT
    x = np.asarray(x, dtype=np.float32)
    w_qkv = np.asarray(w_qkv, dtype=np.float32)
    w_o = np.asarray(w_o, dtype=np.float32)

    nc = _get_nc()
    in_maps = _host_inputs(x, w_qkv, w_o)
    trace = bool(int(os.environ.get("BASS_KERNEL_TRACE", "0")))
    last_exc = None
    for _attempt in range(3):
        try:
            res = run_bass_kernel_spmd(
                nc, in_maps, core_ids=list(range(NC)),
                trace=trace, trace_cores=list(range(NC)) if trace else None)
            break
        except Exception as e:  # transient NRT device errors: retry
            last_exc = e
    else:
        raise last_exc
    LAST_RESULT = res

    out = np.empty((B, S, HID), dtype=np.float32)
    for b in range(B):
        acc = np.zeros((S, HID), dtype=np.float64)
        for g in range(4):
            acc += res.results[b * 4 + g]["out"]
        out[b] = acc.astype(np.float32)
    return out



# revision 6
# speedup vs baseline: 1.0617x; 1.0617x over previous
"""Causal self-attention with RoPE on 8 Trainium2 NeuronCores (v5).

Problem: B=2, S=2048, H=16 heads, D=128, HID=2048, fp32.
  qkv = x @ w_qkv.T ; RoPE(q, k) ; causal softmax(q k^T / sqrt(D)) @ v ; out @ w_o.T

Sharding (hardcoded): core c handles batch b = c // 4 and head group
g = c % 4 (heads 4g..4g+4). Each core computes a partial (S, HID) output
contracted over its 512 hidden dims of the o-projection; the host sums the 4
partials per batch.

All matmuls run in fp32r (TF32-class): at moving dims >=256 fp32r streams
1 column/cycle at full clock; bf16 draws more PE power and downclocks
(~259 ns vs ~227 ns per 512-col matmul, measured).

Engine discipline (engine queues are strict in-order FIFOs, so latency
coupling matters more than busy%):
 - ACT: all PSUM->SBUF copies + exp/ln. DVE: RoPE + mask adds + normalize
   muls only, so a copy never queues behind a stalled RoPE/normalize.
 - The softmax reciprocal is split in three (ln+exp on ACT / broadcast on
   GpSimd / multiply on DVE) and each stage is emitted only at a program
   point where its inputs are already done, so no FIFO head-blocks.
 - Phase C is NOT interleaved into B: C units gate on normalize chains and
   would stall the in-order PE stream (measured regression).

DMA discipline: per-dma_start completion bandwidth is only ~40 GB/s
(aggregate across queues ~330 GB/s), so the first x chunks are split into
column-halves across two rings, weights are host-prepped contiguous, and
issue order matches consumption order. The v projection runs as a single
kc pass so each x chunk's SBUF slot retires early — the next half's x
refills stream in under the whole v pass and the half boundary has no
DMA bubble.
"""

import os

import numpy as np

import concourse.bacc as bacc
import concourse.tile as tile
from concourse import mybir
from concourse.bass_utils import run_bass_kernel_spmd

B, S, H, D = 2, 2048, 16, 128
HID = H * D
THETA = 10000.0
SCALE = 1.0 / float(np.sqrt(D))
NH = 4                 # heads per core
NC = 8                 # cores
NKC = HID // 128       # contraction chunks (128 wide)
SB = 512               # attention si-block / moving dim
NSB = S // SB          # si blocks
SH = S // 2            # s-half
F32 = mybir.dt.float32

MM_MODE = os.environ.get("BASS_MM_MODE", "fp32r")
MMDT = mybir.dt.float32r if MM_MODE == "fp32r" else mybir.dt.float32

LAST_RESULT = None  # BassKernelResults of the most recent run (for test harness)


def _build_nc():
    nc = bacc.Bacc("TRN2", target_bir_lowering=False, debug=False, num_devices=NC)

    xT = nc.dram_tensor("xT", [HID, S], F32, kind="ExternalInput")
    wqk = nc.dram_tensor("wqk", [2 * NH, 128, HID], F32, kind="ExternalInput")
    wv = nc.dram_tensor("wv", [NKC, 128, NH * 128], F32, kind="ExternalInput")
    woT = nc.dram_tensor("woT", [NH * 128, HID], F32, kind="ExternalInput")
    cosT = nc.dram_tensor("cosT", [D, S], F32, kind="ExternalInput")
    sinST = nc.dram_tensor("sinST", [D, S], F32, kind="ExternalInput")
    maskadd = nc.dram_tensor("maskadd", [128, 128], F32, kind="ExternalInput")
    out = nc.dram_tensor("out", [S, HID], F32, kind="ExternalOutput")

    with tile.TileContext(nc) as tc:
        with tc.tile_pool(name="pconst", bufs=1) as pconst, \
             tc.tile_pool(name="pqk", bufs=1) as pqk, \
             tc.tile_pool(name="pvn", bufs=1) as pvn:

            ones_f = pconst.tile([128, 4], F32, name="ones_f")
            nc.vector.memset(ones_f, 1.0)
            ones4 = pconst.tile([128, 4], MMDT, name="ones4")
            nc.vector.tensor_copy(ones4, ones_f)
            tri_t = pconst.tile([128, 128], F32, name="tri")
            nc.scalar.dma_start(out=tri_t, in_=maskadd[:, :])

            qT = [pqk.tile([128, S], MMDT, name=f"qT_{h}") for h in range(NH)]
            kT = [pqk.tile([128, S], MMDT, name=f"kT_{h}") for h in range(NH)]
            vn = [pvn.tile([128, 4, NH * 128], MMDT, name=f"vn_{g}")
                  for g in range(4)]
            outT = qT  # attention output aliases qT per si-block

            # ---- Phase A: q/k/v projection + RoPE, per s-half ----
            with tc.tile_pool(name="px", bufs=1) as px, \
                 tc.tile_pool(name="pwq", bufs=3) as pwq, \
                 tc.tile_pool(name="pwv", bufs=4) as pwvp, \
                 tc.tile_pool(name="ptrig", bufs=1) as ptrig, \
                 tc.tile_pool(name="psh", bufs=1) as psh:
                for half in range(2):
                    s0 = half * SH
                    xh = []
                    for kc in range(NKC):
                        xt = px.tile([128, SH], MMDT, name=f"xh{kc}")
                        src = xT[kc * 128:(kc + 1) * 128, s0:s0 + SH]
                        if half == 0 and kc < 4:
                            # single-DMA completion BW is ~40 GB/s: split the
                            # startup-critical chunks across two rings
                            nc.sync.dma_start(
                                out=xt[:, 0:SB],
                                in_=src[:, 0:SB].bitcast(MMDT))
                            nc.gpsimd.dma_start(
                                out=xt[:, SB:SH],
                                in_=src[:, SB:SH].bitcast(MMDT))
                        else:
                            eng = nc.sync if kc % 2 == 0 else nc.gpsimd
                            eng.dma_start(out=xt, in_=src.bitcast(MMDT))
                        xh.append(xt)
                    cos_t = ptrig.tile([D, SH], F32, name="cosT")
                    sin_t = ptrig.tile([D, SH], F32, name="sinST")
                    nc.scalar.dma_start(out=cos_t, in_=cosT[:, s0:s0 + SH])
                    nc.scalar.dma_start(out=sin_t, in_=sinST[:, s0:s0 + SH])

                    # q/k projection, transposed output [d, s], then RoPE.
                    # All PSUM->SBUF copies on ACT: the DVE FIFO holds only
                    # RoPE ops, so a late cos/sin or shuffle never delays
                    # PSUM bank recycling.
                    ppa_cm = tc.tile_pool(name="ppa", bufs=8, space="PSUM")
                    ppa = ppa_cm.__enter__()
                    for h in range(NH):
                        for kind, dst in ((0, qT[h]), (1, kT[h])):
                            ot = kind * NH + h
                            wt = pwq.tile([128, HID], MMDT, name="wqk")
                            for pc in range(2):
                                nc.scalar.dma_start(
                                    out=wt[:, pc * 1024:(pc + 1) * 1024],
                                    in_=wqk[ot][:, pc * 1024:(pc + 1) * 1024]
                                    .bitcast(MMDT))
                            ps0 = ppa.tile([128, SB], F32, name="qkps")
                            ps1 = ppa.tile([128, SB], F32, name="qkps")
                            for kc in range(NKC):
                                nc.tensor.matmul(
                                    ps0, wt[:, kc * 128:(kc + 1) * 128],
                                    xh[kc][:, 0:SB],
                                    start=(kc == 0), stop=(kc == NKC - 1))
                                nc.tensor.matmul(
                                    ps1, wt[:, kc * 128:(kc + 1) * 128],
                                    xh[kc][:, SB:SH],
                                    start=(kc == 0), stop=(kc == NKC - 1))
                            nc.scalar.copy(out=dst[:, s0:s0 + SB], in_=ps0)
                            nc.scalar.copy(out=dst[:, s0 + SB:s0 + SH], in_=ps1)
                            # RoPE in place (rotate-half partition swap by DMA)
                            sl = dst[:, s0:s0 + SH]
                            sh_t = psh.tile([128, SH], MMDT, name="shuf")
                            nc.gpsimd.dma_start(out=sh_t[0:64, :],
                                                in_=dst[64:128, s0:s0 + SH])
                            nc.gpsimd.dma_start(out=sh_t[64:128, :],
                                                in_=dst[0:64, s0:s0 + SH])
                            nc.vector.tensor_mul(sh_t, sh_t, sin_t)
                            nc.vector.tensor_mul(sl, sl, cos_t)
                            nc.vector.tensor_add(sl, sl, sh_t)
                    ppa_cm.__exit__(None, None, None)

                    # v projection, natural layout [s, 4 heads x d]: one kc
                    # pass over 8 PSUM banks, so each x chunk's slot retires
                    # at its kc step and the next half's refill overlaps the
                    # whole pass
                    pvp_cm = tc.tile_pool(name="pvp", bufs=8, space="PSUM")
                    pvp = pvp_cm.__enter__()
                    wv_t = []
                    for kc in range(NKC):
                        wvt = pwvp.tile([128, NH * 128], MMDT, name="wv")
                        nc.scalar.dma_start(out=wvt, in_=wv[kc].bitcast(MMDT))
                        wv_t.append(wvt)
                    vps = [pvp.tile([128, NH * 128], F32, name="vps")
                           for _ in range(8)]
                    for kc in range(NKC):
                        for st in range(8):
                            nc.tensor.matmul(
                                vps[st], xh[kc][:, st * 128:(st + 1) * 128],
                                wv_t[kc],
                                start=(kc == 0), stop=(kc == NKC - 1))
                    for st in range(8):
                        sg = half * 8 + st   # global s-chunk
                        nc.scalar.copy(out=vn[sg // 4][:, sg % 4, :],
                                       in_=vps[st])
                    pvp_cm.__exit__(None, None, None)

            # o-proj weights: issued at B start, resident long before C
            pwo_cm = tc.tile_pool(name="pwo", bufs=1)
            pwo = pwo_cm.__enter__()
            wo_t = []
            for h in range(NH):
                wot = pwo.tile([128, HID], MMDT, name=f"wo{h}")
                nc.scalar.dma_start(
                    out=wot,
                    in_=woT[h * 128:(h + 1) * 128, :].bitcast(MMDT))
                wo_t.append(wot)

            # ---- Phase B: attention per (si-block round, head) ----
            # Softmax denominator: e-tiles are accumulated on DVE/GpSimd into
            # e_acc (instead of streaming every chunk through a [128,4]-ones
            # matmul -> removes ~70K columns from the tensor stream); one
            # 512-col ones-matmul per (h, sib) reduces e_acc partitions, then
            # DVE reciprocal_approx_fast (18 bits) replaces the ACT ln/exp
            # pair. The 4 stages (ones-mm -> rec -> broadcast -> normalize
            # mul) advance one step per chunk-pair slot so every op reaches
            # its in-order FIFO only when its inputs are already done.
            pending = []   # [stage, h, si0, o_ps, (pair, lo, start), l4, rb]

            with tc.tile_pool(name="pexp", bufs=4) as pexp, \
                 tc.tile_pool(name="ppr", bufs=4) as ppr, \
                 tc.tile_pool(name="prr", bufs=2) as prr, \
                 tc.tile_pool(name="prb", bufs=2) as prb, \
                 tc.tile_pool(name="psc", bufs=2, space="PSUM") as psc, \
                 tc.tile_pool(name="plp", bufs=1, space="PSUM") as plp, \
                 tc.tile_pool(name="pop", bufs=3, space="PSUM") as pop:

                def advance(item):
                    st = item[0]
                    _, h, si0, o_ps, last_pair, l4, rb = item
                    if st == 0:
                        # final ones-matmul of this head's denominator
                        pr, lo, first = last_pair
                        nc.tensor.matmul(l4[:, lo:], ones4, pr[:, lo:],
                                         start=first, stop=True)
                    elif st == 1:
                        rec = prr.tile([1, SB], F32, name="rec")
                        nc.vector.reciprocal_approx_fast(
                            out=rec, in_=l4[0:1, :])
                        rb = prb.tile([128, SB], F32, name="rb")
                        nc.gpsimd.partition_broadcast(rb, rec)
                        item[6] = rb
                    else:
                        nc.vector.tensor_mul(outT[h][:, si0:si0 + SB],
                                             o_ps, rb)
                    item[0] += 1
                    return item[0] >= 3

                def slot(budget):
                    done = 0
                    for item in list(pending):
                        if done >= budget:
                            break
                        if advance(item):
                            pending.remove(item)
                        done += 1

                for sib in range(NSB):
                    si0 = sib * SB
                    nch = 4 * (sib + 1)
                    for h in range(NH):
                        o_ps = pop.tile([128, SB], F32, name="ops")
                        l4 = plp.tile([4, SB], F32, name="l4")
                        prev_pair = None
                        for cp in range(nch // 2):
                            s_ps = psc.tile([128, 2, SB], F32, name="sps")
                            e_t = pexp.tile([128, 2, SB], MMDT, name="exp")
                            los = []
                            for j in range(2):
                                cj = cp * 2 + j
                                dg = cj - (nch - 4)
                                lo = dg * 128 if dg > 0 else 0
                                los.append((cj, lo))
                                nc.tensor.matmul(
                                    s_ps[:, j, lo:],
                                    kT[h][:, cj * 128:(cj + 1) * 128],
                                    qT[h][:, si0 + lo:si0 + SB],
                                    start=True, stop=True)
                                if dg >= 0:
                                    nc.vector.tensor_add(
                                        s_ps[:, j, lo:lo + 128],
                                        s_ps[:, j, lo:lo + 128], tri_t)
                            # exp only over the valid [lo:] ranges (the stale
                            # sub-diagonal region was ~15% wasted ACT time)
                            if los[0][1] == 0 and los[1][1] == 0:
                                nc.scalar.activation(
                                    out=e_t, in_=s_ps,
                                    func=mybir.ActivationFunctionType.Exp,
                                    scale=SCALE)
                            else:
                                for j in range(2):
                                    lo = los[j][1]
                                    nc.scalar.activation(
                                        out=e_t[:, j, lo:],
                                        in_=s_ps[:, j, lo:],
                                        func=mybir.ActivationFunctionType.Exp,
                                        scale=SCALE)
                            # previous heads' deferred normalize stages
                            slot(2 if sib == 0 else 1)
                            # independent pair-sum of the two e tiles (no
                            # serial chain; DVE/GpSimd alternate by parity)
                            lo0, lo1 = los[0][1], los[1][1]
                            pr = ppr.tile([128, SB], MMDT, name="pair")
                            eng = nc.vector if cp % 2 == 0 else nc.gpsimd
                            if lo0 == lo1:
                                eng.tensor_add(pr[:, lo0:], e_t[:, 0, lo0:],
                                               e_t[:, 1, lo1:])
                            else:
                                eng.tensor_copy(pr[:, lo0:lo1],
                                                e_t[:, 0, lo0:lo1])
                                eng.tensor_add(pr[:, lo1:], e_t[:, 0, lo1:],
                                               e_t[:, 1, lo1:])
                            # ones-matmul for the PREVIOUS pair (one cp of
                            # slack so the tensor FIFO never waits on it)
                            if prev_pair is not None:
                                ppr_t, plo, first = prev_pair
                                nc.tensor.matmul(
                                    l4[:, plo:], ones4, ppr_t[:, plo:],
                                    start=first, stop=False)
                            prev_pair = (pr, lo0, cp == 0)
                            for j in range(2):
                                cj, lo = los[j]
                                nc.tensor.matmul(
                                    o_ps[:, lo:],
                                    vn[cj // 4][:, cj % 4,
                                                h * 128:(h + 1) * 128],
                                    e_t[:, j, lo:],
                                    start=(cj == 0), stop=(cj == nch - 1))
                        pending.append([0, h, si0, o_ps, prev_pair, l4, None])
                    # round end: drain so at most one head stays in flight
                    while len(pending) > 1:
                        if advance(pending[0]):
                            pending.pop(0)
                while pending:
                    if advance(pending[0]):
                        pending.pop(0)

            # ---- Phase C: partial o-projection ----
            with tc.tile_pool(name="pft", bufs=4, space="PSUM") as pft, \
                 tc.tile_pool(name="pst", bufs=4) as pst:
                u = 0
                for st in range(S // 128):
                    for ob in range(HID // SB):
                        fin = pft.tile([128, SB], F32, name="fin")
                        for hh in range(NH):
                            nc.tensor.matmul(
                                fin, outT[hh][:, st * 128:(st + 1) * 128],
                                wo_t[hh][:, ob * SB:(ob + 1) * SB],
                                start=(hh == 0), stop=(hh == NH - 1))
                        stg = pst.tile([128, SB], F32, name="stg")
                        if u % 2 == 0:
                            nc.scalar.copy(out=stg, in_=fin)
                        else:
                            nc.vector.tensor_copy(stg, fin)
                        # alternate output rings so the final drain halves
                        eng = nc.sync if u % 2 == 0 else nc.gpsimd
                        u += 1
                        eng.dma_start(
                            out=out[st * 128:(st + 1) * 128,
                                    ob * SB:(ob + 1) * SB],
                            in_=stg)

            pwo_cm.__exit__(None, None, None)

    # Force exp and ln onto the single `natural_log_exp_and_others` ACT
    # table set: with the default map the table-load pass alternates between
    # the exp-only and ln-only sets (~2.7us per reload on ScalarE). Blank
    # the single-function sets (positions preserved, so set ids stay valid)
    # so both functions resolve to the combined set -> one load.
    import concourse.bacc as _bacc_mod
    import concourse.hw_specs as _hw_specs
    _orig_tables = _hw_specs.get_activation_tables

    def _patched_tables(arch):
        t = dict(_orig_tables(arch))
        for name in ("exp_and_others", "exp_and_friends", "natural_log"):
            if name in t:
                t[name] = set()
        return t

    _bacc_mod.get_activation_tables = _patched_tables
    try:
        nc.compile()
    finally:
        _bacc_mod.get_activation_tables = _orig_tables
    return nc


_NC_CACHE = None


def _get_nc():
    global _NC_CACHE
    if _NC_CACHE is None:
        _NC_CACHE = _build_nc()
    return _NC_CACHE


def _host_inputs(x, w_qkv, w_o):
    """Per-core input maps (sharding + contiguous-DMA layout prep on host)."""
    inv_freq = 1.0 / (THETA ** (np.arange(0, D, 2, dtype=np.float64) / D))
    pos = np.arange(S, dtype=np.float64)
    freqs = pos[:, None] * inv_freq[None, :]          # (S, D/2)
    emb = np.concatenate([freqs, freqs], axis=-1)     # (S, D)
    cosT = np.ascontiguousarray(np.cos(emb).T.astype(np.float32))   # (D, S)
    sign = np.concatenate([-np.ones(D // 2), np.ones(D // 2)])
    sinST = np.ascontiguousarray((sign[None, :] * np.sin(emb)).T
                                 .astype(np.float32))               # (D, S)
    # additive causal triangle for a diagonal 128x128 block of scores^T:
    # keep (add 0) when sj_local <= si_local, else -1e30
    p = np.arange(128)[:, None]
    f = np.arange(128)[None, :]
    maskadd = np.where(p <= f, 0.0, -1e30).astype(np.float32)       # (128, 128)

    xTb = [np.ascontiguousarray(x[b].T) for b in range(B)]          # (HID, S)
    in_maps = []
    for c in range(NC):
        b, g = c // 4, c % 4
        rows = slice(g * NH * D, (g + 1) * NH * D)
        wq = w_qkv[0 * HID:1 * HID][rows]             # (512, 2048)
        wk = w_qkv[1 * HID:2 * HID][rows]
        wvm = w_qkv[2 * HID:3 * HID][rows]
        # wqk[ot][p, kc*128+od] = w[ot*128+od, kc*128+p]
        wqk_arr = np.empty((2 * NH, 128, HID), dtype=np.float32)
        for kind, wm in ((0, wq), (1, wk)):
            for h in range(NH):
                wT = wm[h * 128:(h + 1) * 128].T      # (2048 hid, 128 od)
                wqk_arr[kind * NH + h] = (
                    wT.reshape(NKC, 128, 128).transpose(1, 0, 2)
                    .reshape(128, HID))
        # wv[kc][p, j] = w_v[j, kc*128+p]
        wv_arr = np.ascontiguousarray(
            wvm.T.reshape(NKC, 128, NH * 128)).astype(np.float32)
        woT = np.ascontiguousarray(w_o[:, rows].T).astype(np.float32)
        in_maps.append({
            "xT": xTb[b], "wqk": wqk_arr, "wv": wv_arr, "woT": woT,
            "cosT": cosT, "sinST": sinST, "maskadd": maskadd,
        })
    return in_maps


def kernel(x, w_qkv, w_o):
    global LAST_RESULT
    x = np.asarray(x, dtype=np.float32)
    w_qkv = np.asarray(w_qkv, dtype=np.float32)
    w_o = np.asarray(w_o, dtype=np.float32)

    nc = _get_nc()
    in_maps = _host_inputs(x, w_qkv, w_o)
    trace = bool(int(os.environ.get("BASS_KERNEL_TRACE", "0")))
    last_exc = None
    for _attempt in range(3):
        try:
            res = run_bass_kernel_spmd(
                nc, in_maps, core_ids=list(range(NC)),
                trace=trace, trace_cores=list(range(NC)) if trace else None)
            break
        except Exception as e:  # transient NRT device errors: retry
            last_exc = e
    else:
        raise last_exc
    LAST_RESULT = res

    out = np.empty((B, S, HID), dtype=np.float32)
    for b in range(B):
        acc = np.zeros((S, HID), dtype=np.float64)
        for g in range(4):
            acc += res.results[b * 4 + g]["out"]
        out[b] = acc.astype(np.float32)
    return out



# revision 7
# speedup vs baseline: 1.4744x; 1.3887x over previous
"""Causal self-attention with RoPE on 8 Trainium2 NeuronCores (v5).

Problem: B=2, S=2048, H=16 heads, D=128, HID=2048, fp32.
  qkv = x @ w_qkv.T ; RoPE(q, k) ; causal softmax(q k^T / sqrt(D)) @ v ; out @ w_o.T

Sharding (hardcoded): core c handles batch b = c // 4 and head group
g = c % 4 (heads 4g..4g+4). Each core computes a partial (S, HID) output
contracted over its 512 hidden dims of the o-projection; the host sums the 4
partials per batch.

All matmuls run in fp32r (TF32-class): at moving dims >=256 fp32r streams
1 column/cycle at full clock; bf16 draws more PE power and downclocks
(~259 ns vs ~227 ns per 512-col matmul, measured).

Engine discipline (engine queues are strict in-order FIFOs, so latency
coupling matters more than busy%):
 - ACT: all PSUM->SBUF copies + exp/ln. DVE: RoPE + mask adds + normalize
   muls only, so a copy never queues behind a stalled RoPE/normalize.
 - The softmax reciprocal is split in three (ln+exp on ACT / broadcast on
   GpSimd / multiply on DVE) and each stage is emitted only at a program
   point where its inputs are already done, so no FIFO head-blocks.
 - Phase C is NOT interleaved into B: C units gate on normalize chains and
   would stall the in-order PE stream (measured regression).

DMA discipline: per-dma_start completion bandwidth is only ~40 GB/s
(aggregate across queues ~330 GB/s), so the first x chunks are split into
column-halves across two rings, weights are host-prepped contiguous, and
issue order matches consumption order. The v projection runs as a single
kc pass so each x chunk's SBUF slot retires early — the next half's x
refills stream in under the whole v pass and the half boundary has no
DMA bubble.
"""

import os

import numpy as np

import concourse.bacc as bacc
import concourse.tile as tile
from concourse import mybir
from concourse.bass_utils import run_bass_kernel_spmd

B, S, H, D = 2, 2048, 16, 128
HID = H * D
THETA = 10000.0
SCALE = 1.0 / float(np.sqrt(D))
NH = 4                 # heads per core
NC = 8                 # cores
NKC = HID // 128       # contraction chunks (128 wide)
SB = 512               # attention si-block / moving dim
NSB = S // SB          # si blocks
SH = S // 2            # s-half
F32 = mybir.dt.float32

MM_MODE = os.environ.get("BASS_MM_MODE", "fp32r")
MMDT = mybir.dt.float32r if MM_MODE == "fp32r" else mybir.dt.float32

LAST_RESULT = None  # BassKernelResults of the most recent run (for test harness)


def _build_nc():
    nc = bacc.Bacc("TRN2", target_bir_lowering=False, debug=False, num_devices=NC)

    xT = nc.dram_tensor("xT", [HID, S], F32, kind="ExternalInput")
    wqk = nc.dram_tensor("wqk", [2 * NH, 128, HID], F32, kind="ExternalInput")
    wv = nc.dram_tensor("wv", [NKC, 128, NH * 128], F32, kind="ExternalInput")
    woT = nc.dram_tensor("woT", [NH * 128, HID], F32, kind="ExternalInput")
    cosT = nc.dram_tensor("cosT", [D, S], F32, kind="ExternalInput")
    sinST = nc.dram_tensor("sinST", [D, S], F32, kind="ExternalInput")
    maskadd = nc.dram_tensor("maskadd", [128, 128], F32, kind="ExternalInput")
    out = nc.dram_tensor("out", [S, HID], F32, kind="ExternalOutput")

    with tile.TileContext(nc) as tc:
        with tc.tile_pool(name="pconst", bufs=1) as pconst, \
             tc.tile_pool(name="pqk", bufs=1) as pqk, \
             tc.tile_pool(name="pvn", bufs=1) as pvn:

            ones_f = pconst.tile([128, 4], F32, name="ones_f")
            nc.vector.memset(ones_f, 1.0)
            ones4 = pconst.tile([128, 4], MMDT, name="ones4")
            nc.vector.tensor_copy(ones4, ones_f)
            tri_t = pconst.tile([128, 128], F32, name="tri")
            nc.scalar.dma_start(out=tri_t, in_=maskadd[:, :])

            qT = [pqk.tile([128, S], MMDT, name=f"qT_{h}") for h in range(NH)]
            kT = [pqk.tile([128, S], MMDT, name=f"kT_{h}") for h in range(NH)]
            vn = [pvn.tile([128, 4, NH * 128], MMDT, name=f"vn_{g}")
                  for g in range(4)]
            outT = qT  # attention output aliases qT per si-block

            # ---- Phase A: q/k/v projection + RoPE, per s-half ----
            with tc.tile_pool(name="px", bufs=1) as px, \
                 tc.tile_pool(name="pwq", bufs=3) as pwq, \
                 tc.tile_pool(name="pwv", bufs=4) as pwvp, \
                 tc.tile_pool(name="ptrig", bufs=1) as ptrig, \
                 tc.tile_pool(name="psh", bufs=1) as psh:
                for half in range(2):
                    s0 = half * SH
                    xh = []
                    for kc in range(NKC):
                        xt = px.tile([128, SH], MMDT, name=f"xh{kc}")
                        src = xT[kc * 128:(kc + 1) * 128, s0:s0 + SH]
                        if half == 0 and kc < 4:
                            # single-DMA completion BW is ~40 GB/s: split the
                            # startup-critical chunks across two rings
                            nc.sync.dma_start(
                                out=xt[:, 0:SB],
                                in_=src[:, 0:SB].bitcast(MMDT))
                            nc.gpsimd.dma_start(
                                out=xt[:, SB:SH],
                                in_=src[:, SB:SH].bitcast(MMDT))
                        else:
                            eng = nc.sync if kc % 2 == 0 else nc.gpsimd
                            eng.dma_start(out=xt, in_=src.bitcast(MMDT))
                        xh.append(xt)
                    cos_t = ptrig.tile([D, SH], F32, name="cosT")
                    sin_t = ptrig.tile([D, SH], F32, name="sinST")
                    nc.scalar.dma_start(out=cos_t, in_=cosT[:, s0:s0 + SH])
                    nc.scalar.dma_start(out=sin_t, in_=sinST[:, s0:s0 + SH])

                    # q/k projection, transposed output [d, s], then RoPE.
                    # All PSUM->SBUF copies on ACT: the DVE FIFO holds only
                    # RoPE ops, so a late cos/sin or shuffle never delays
                    # PSUM bank recycling.
                    ppa_cm = tc.tile_pool(name="ppa", bufs=8, space="PSUM")
                    ppa = ppa_cm.__enter__()
                    for h in range(NH):
                        for kind, dst in ((0, qT[h]), (1, kT[h])):
                            ot = kind * NH + h
                            wt = pwq.tile([128, HID], MMDT, name="wqk")
                            for pc in range(2):
                                nc.scalar.dma_start(
                                    out=wt[:, pc * 1024:(pc + 1) * 1024],
                                    in_=wqk[ot][:, pc * 1024:(pc + 1) * 1024]
                                    .bitcast(MMDT))
                            ps0 = ppa.tile([128, SB], F32, name="qkps")
                            ps1 = ppa.tile([128, SB], F32, name="qkps")
                            for kc in range(NKC):
                                nc.tensor.matmul(
                                    ps0, wt[:, kc * 128:(kc + 1) * 128],
                                    xh[kc][:, 0:SB],
                                    start=(kc == 0), stop=(kc == NKC - 1))
                                nc.tensor.matmul(
                                    ps1, wt[:, kc * 128:(kc + 1) * 128],
                                    xh[kc][:, SB:SH],
                                    start=(kc == 0), stop=(kc == NKC - 1))
                            nc.scalar.copy(out=dst[:, s0:s0 + SB], in_=ps0)
                            nc.scalar.copy(out=dst[:, s0 + SB:s0 + SH], in_=ps1)
                            # RoPE in place (rotate-half partition swap by DMA)
                            sl = dst[:, s0:s0 + SH]
                            sh_t = psh.tile([128, SH], MMDT, name="shuf")
                            nc.gpsimd.dma_start(out=sh_t[0:64, :],
                                                in_=dst[64:128, s0:s0 + SH])
                            nc.gpsimd.dma_start(out=sh_t[64:128, :],
                                                in_=dst[0:64, s0:s0 + SH])
                            nc.vector.tensor_mul(sh_t, sh_t, sin_t)
                            nc.vector.tensor_mul(sl, sl, cos_t)
                            nc.vector.tensor_add(sl, sl, sh_t)
                    ppa_cm.__exit__(None, None, None)

                    # v projection, natural layout [s, 4 heads x d]: one kc
                    # pass over 8 PSUM banks, so each x chunk's slot retires
                    # at its kc step and the next half's refill overlaps the
                    # whole pass
                    pvp_cm = tc.tile_pool(name="pvp", bufs=8, space="PSUM")
                    pvp = pvp_cm.__enter__()
                    wv_t = []
                    for kc in range(NKC):
                        wvt = pwvp.tile([128, NH * 128], MMDT, name="wv")
                        nc.scalar.dma_start(out=wvt, in_=wv[kc].bitcast(MMDT))
                        wv_t.append(wvt)
                    vps = [pvp.tile([128, NH * 128], F32, name="vps")
                           for _ in range(8)]
                    for kc in range(NKC):
                        for st in range(8):
                            nc.tensor.matmul(
                                vps[st], xh[kc][:, st * 128:(st + 1) * 128],
                                wv_t[kc],
                                start=(kc == 0), stop=(kc == NKC - 1))
                    for st in range(8):
                        sg = half * 8 + st   # global s-chunk
                        nc.scalar.copy(out=vn[sg // 4][:, sg % 4, :],
                                       in_=vps[st])
                    pvp_cm.__exit__(None, None, None)

            # o-proj weights: issued at B start, resident long before C
            pwo_cm = tc.tile_pool(name="pwo", bufs=1)
            pwo = pwo_cm.__enter__()
            wo_t = []
            for h in range(NH):
                wot = pwo.tile([128, HID], MMDT, name=f"wo{h}")
                nc.scalar.dma_start(
                    out=wot,
                    in_=woT[h * 128:(h + 1) * 128, :].bitcast(MMDT))
                wo_t.append(wot)

            # ---- Phase B: attention per (si-block round, head) ----
            # Softmax denominator: e-tiles are accumulated on DVE/GpSimd into
            # e_acc (instead of streaming every chunk through a [128,4]-ones
            # matmul -> removes ~70K columns from the tensor stream); one
            # 512-col ones-matmul per (h, sib) reduces e_acc partitions, then
            # DVE reciprocal_approx_fast (18 bits) replaces the ACT ln/exp
            # pair. The 4 stages (ones-mm -> rec -> broadcast -> normalize
            # mul) advance one step per chunk-pair slot so every op reaches
            # its in-order FIFO only when its inputs are already done.
            pending = []   # [stage, h, si0, o_ps, (pair, lo, start), l4, rb]

            with tc.tile_pool(name="pexp", bufs=4) as pexp, \
                 tc.tile_pool(name="ppr", bufs=4) as ppr, \
                 tc.tile_pool(name="prr", bufs=2) as prr, \
                 tc.tile_pool(name="prb", bufs=2) as prb, \
                 tc.tile_pool(name="psc", bufs=2, space="PSUM") as psc, \
                 tc.tile_pool(name="plp", bufs=1, space="PSUM") as plp, \
                 tc.tile_pool(name="pop", bufs=3, space="PSUM") as pop:

                def advance(item):
                    st = item[0]
                    _, h, si0, o_ps, last_pair, l4, rb = item
                    if st == 0:
                        # final ones-matmul of this head's denominator
                        pr, lo, first = last_pair
                        nc.tensor.matmul(l4[:, lo:], ones4, pr[:, lo:],
                                         start=first, stop=True)
                    elif st == 1:
                        rec = prr.tile([1, SB], F32, name="rec")
                        nc.vector.reciprocal_approx_fast(
                            out=rec, in_=l4[0:1, :])
                        rb = prb.tile([128, SB], F32, name="rb")
                        nc.gpsimd.partition_broadcast(rb, rec)
                        item[6] = rb
                    else:
                        nc.vector.tensor_mul(outT[h][:, si0:si0 + SB],
                                             o_ps, rb)
                    item[0] += 1
                    return item[0] >= 3

                def slot(budget):
                    done = 0
                    for item in list(pending):
                        if done >= budget:
                            break
                        if advance(item):
                            pending.remove(item)
                        done += 1

                for sib in range(NSB):
                    si0 = sib * SB
                    nch = 4 * (sib + 1)
                    for h in range(NH):
                        o_ps = pop.tile([128, SB], F32, name="ops")
                        l4 = plp.tile([4, SB], F32, name="l4")
                        prev_pair = None
                        for cp in range(nch // 2):
                            s_ps = psc.tile([128, 2, SB], F32, name="sps")
                            e_t = pexp.tile([128, 2, SB], MMDT, name="exp")
                            los = []
                            for j in range(2):
                                cj = cp * 2 + j
                                dg = cj - (nch - 4)
                                lo = dg * 128 if dg > 0 else 0
                                los.append((cj, lo))
                                nc.tensor.matmul(
                                    s_ps[:, j, lo:],
                                    kT[h][:, cj * 128:(cj + 1) * 128],
                                    qT[h][:, si0 + lo:si0 + SB],
                                    start=True, stop=True)
                                if dg >= 0:
                                    nc.vector.tensor_add(
                                        s_ps[:, j, lo:lo + 128],
                                        s_ps[:, j, lo:lo + 128], tri_t)
                            # exp only over the valid [lo:] ranges (the stale
                            # sub-diagonal region was ~15% wasted ACT time)
                            if los[0][1] == 0 and los[1][1] == 0:
                                nc.scalar.activation(
                                    out=e_t, in_=s_ps,
                                    func=mybir.ActivationFunctionType.Exp,
                                    scale=SCALE)
                            else:
                                for j in range(2):
                                    lo = los[j][1]
                                    nc.scalar.activation(
                                        out=e_t[:, j, lo:],
                                        in_=s_ps[:, j, lo:],
                                        func=mybir.ActivationFunctionType.Exp,
                                        scale=SCALE)
                            # previous heads' deferred normalize stages
                            slot(2 if sib == 0 else 1)
                            # independent pair-sum of the two e tiles. DVE
                            # ONLY: gpsimd tensor ops swap in a DSP library
                            # (MODIFY_POOL_CONFIG LOAD_LIB ~6us) evicting the
                            # partition_broadcast lib -> measured disaster.
                            lo0, lo1 = los[0][1], los[1][1]
                            pr = ppr.tile([128, SB], MMDT, name="pair")
                            if lo0 == lo1:
                                nc.vector.tensor_add(pr[:, lo0:],
                                                     e_t[:, 0, lo0:],
                                                     e_t[:, 1, lo1:])
                            else:
                                nc.vector.tensor_copy(pr[:, lo0:lo1],
                                                      e_t[:, 0, lo0:lo1])
                                nc.vector.tensor_add(pr[:, lo1:],
                                                     e_t[:, 0, lo1:],
                                                     e_t[:, 1, lo1:])
                            # ones-matmul for the PREVIOUS pair (one cp of
                            # slack so the tensor FIFO never waits on it)
                            if prev_pair is not None:
                                ppr_t, plo, first = prev_pair
                                nc.tensor.matmul(
                                    l4[:, plo:], ones4, ppr_t[:, plo:],
                                    start=first, stop=False)
                            prev_pair = (pr, lo0, cp == 0)
                            for j in range(2):
                                cj, lo = los[j]
                                nc.tensor.matmul(
                                    o_ps[:, lo:],
                                    vn[cj // 4][:, cj % 4,
                                                h * 128:(h + 1) * 128],
                                    e_t[:, j, lo:],
                                    start=(cj == 0), stop=(cj == nch - 1))
                        pending.append([0, h, si0, o_ps, prev_pair, l4, None])
                    # round end: drain so at most one head stays in flight
                    while len(pending) > 1:
                        if advance(pending[0]):
                            pending.pop(0)
                while pending:
                    if advance(pending[0]):
                        pending.pop(0)

            # ---- Phase C: partial o-projection ----
            with tc.tile_pool(name="pft", bufs=4, space="PSUM") as pft, \
                 tc.tile_pool(name="pst", bufs=4) as pst:
                u = 0
                for st in range(S // 128):
                    for ob in range(HID // SB):
                        fin = pft.tile([128, SB], F32, name="fin")
                        for hh in range(NH):
                            nc.tensor.matmul(
                                fin, outT[hh][:, st * 128:(st + 1) * 128],
                                wo_t[hh][:, ob * SB:(ob + 1) * SB],
                                start=(hh == 0), stop=(hh == NH - 1))
                        stg = pst.tile([128, SB], F32, name="stg")
                        if u % 2 == 0:
                            nc.scalar.copy(out=stg, in_=fin)
                        else:
                            nc.vector.tensor_copy(stg, fin)
                        # alternate output rings so the final drain halves
                        eng = nc.sync if u % 2 == 0 else nc.gpsimd
                        u += 1
                        eng.dma_start(
                            out=out[st * 128:(st + 1) * 128,
                                    ob * SB:(ob + 1) * SB],
                            in_=stg)

            pwo_cm.__exit__(None, None, None)

    # Force exp and ln onto the single `natural_log_exp_and_others` ACT
    # table set: with the default map the table-load pass alternates between
    # the exp-only and ln-only sets (~2.7us per reload on ScalarE). Blank
    # the single-function sets (positions preserved, so set ids stay valid)
    # so both functions resolve to the combined set -> one load.
    import concourse.bacc as _bacc_mod
    import concourse.hw_specs as _hw_specs
    _orig_tables = _hw_specs.get_activation_tables

    def _patched_tables(arch):
        t = dict(_orig_tables(arch))
        for name in ("exp_and_others", "exp_and_friends", "natural_log"):
            if name in t:
                t[name] = set()
        return t

    _bacc_mod.get_activation_tables = _patched_tables
    try:
        nc.compile()
    finally:
        _bacc_mod.get_activation_tables = _orig_tables
    return nc


_NC_CACHE = None


def _get_nc():
    global _NC_CACHE
    if _NC_CACHE is None:
        _NC_CACHE = _build_nc()
    return _NC_CACHE


def _host_inputs(x, w_qkv, w_o):
    """Per-core input maps (sharding + contiguous-DMA layout prep on host)."""
    inv_freq = 1.0 / (THETA ** (np.arange(0, D, 2, dtype=np.float64) / D))
    pos = np.arange(S, dtype=np.float64)
    freqs = pos[:, None] * inv_freq[None, :]          # (S, D/2)
    emb = np.concatenate([freqs, freqs], axis=-1)     # (S, D)
    cosT = np.ascontiguousarray(np.cos(emb).T.astype(np.float32))   # (D, S)
    sign = np.concatenate([-np.ones(D // 2), np.ones(D // 2)])
    sinST = np.ascontiguousarray((sign[None, :] * np.sin(emb)).T
                                 .astype(np.float32))               # (D, S)
    # additive causal triangle for a diagonal 128x128 block of scores^T:
    # keep (add 0) when sj_local <= si_local, else -1e30
    p = np.arange(128)[:, None]
    f = np.arange(128)[None, :]
    maskadd = np.where(p <= f, 0.0, -1e30).astype(np.float32)       # (128, 128)

    xTb = [np.ascontiguousarray(x[b].T) for b in range(B)]          # (HID, S)
    in_maps = []
    for c in range(NC):
        b, g = c // 4, c % 4
        rows = slice(g * NH * D, (g + 1) * NH * D)
        wq = w_qkv[0 * HID:1 * HID][rows]             # (512, 2048)
        wk = w_qkv[1 * HID:2 * HID][rows]
        wvm = w_qkv[2 * HID:3 * HID][rows]
        # wqk[ot][p, kc*128+od] = w[ot*128+od, kc*128+p]
        wqk_arr = np.empty((2 * NH, 128, HID), dtype=np.float32)
        for kind, wm in ((0, wq), (1, wk)):
            for h in range(NH):
                wT = wm[h * 128:(h + 1) * 128].T      # (2048 hid, 128 od)
                wqk_arr[kind * NH + h] = (
                    wT.reshape(NKC, 128, 128).transpose(1, 0, 2)
                    .reshape(128, HID))
        # wv[kc][p, j] = w_v[j, kc*128+p]
        wv_arr = np.ascontiguousarray(
            wvm.T.reshape(NKC, 128, NH * 128)).astype(np.float32)
        woT = np.ascontiguousarray(w_o[:, rows].T).astype(np.float32)
        in_maps.append({
            "xT": xTb[b], "wqk": wqk_arr, "wv": wv_arr, "woT": woT,
            "cosT": cosT, "sinST": sinST, "maskadd": maskadd,
        })
    return in_maps


def kernel(x, w_qkv, w_o):
    global LAST_RESULT
    x = np.asarray(x, dtype=np.float32)
    w_qkv = np.asarray(w_qkv, dtype=np.float32)
    w_o = np.asarray(w_o, dtype=np.float32)

    nc = _get_nc()
    in_maps = _host_inputs(x, w_qkv, w_o)
    trace = bool(int(os.environ.get("BASS_KERNEL_TRACE", "0")))
    last_exc = None
    for _attempt in range(3):
        try:
            res = run_bass_kernel_spmd(
                nc, in_maps, core_ids=list(range(NC)),
                trace=trace, trace_cores=list(range(NC)) if trace else None)
            break
        except Exception as e:  # transient NRT device errors: retry
            last_exc = e
    else:
        raise last_exc
    LAST_RESULT = res

    out = np.empty((B, S, HID), dtype=np.float32)
    for b in range(B):
        acc = np.zeros((S, HID), dtype=np.float64)
        for g in range(4):
            acc += res.results[b * 4 + g]["out"]
        out[b] = acc.astype(np.float32)
    return out



# revision 13
# speedup vs baseline: 1.4953x; 1.0141x over previous
"""Causal self-attention with RoPE on 8 Trainium2 NeuronCores (v5).

Problem: B=2, S=2048, H=16 heads, D=128, HID=2048, fp32.
  qkv = x @ w_qkv.T ; RoPE(q, k) ; causal softmax(q k^T / sqrt(D)) @ v ; out @ w_o.T

Sharding (hardcoded): core c handles batch b = c // 4 and head group
g = c % 4 (heads 4g..4g+4). Each core computes a partial (S, HID) output
contracted over its 512 hidden dims of the o-projection; the host sums the 4
partials per batch.

All matmuls run in fp32r (TF32-class): at moving dims >=256 fp32r streams
1 column/cycle at full clock; bf16 draws more PE power and downclocks
(~259 ns vs ~227 ns per 512-col matmul, measured).

Engine discipline (engine queues are strict in-order FIFOs, so latency
coupling matters more than busy%):
 - ACT: all PSUM->SBUF copies + exp/ln. DVE: RoPE + mask adds + normalize
   muls only, so a copy never queues behind a stalled RoPE/normalize.
 - The softmax reciprocal is split in three (ln+exp on ACT / broadcast on
   GpSimd / multiply on DVE) and each stage is emitted only at a program
   point where its inputs are already done, so no FIFO head-blocks.
 - Phase C is NOT interleaved into B: C units gate on normalize chains and
   would stall the in-order PE stream (measured regression).

DMA discipline: per-dma_start completion bandwidth is only ~40 GB/s
(aggregate across queues ~330 GB/s), so the first x chunks are split into
column-halves across two rings, weights are host-prepped contiguous, and
issue order matches consumption order. The v projection runs as a single
kc pass so each x chunk's SBUF slot retires early — the next half's x
refills stream in under the whole v pass and the half boundary has no
DMA bubble.
"""

import os

import numpy as np

import concourse.bacc as bacc
import concourse.tile as tile
from concourse import mybir
from concourse.bass_utils import run_bass_kernel_spmd

B, S, H, D = 2, 2048, 16, 128
HID = H * D
THETA = 10000.0
SCALE = 1.0 / float(np.sqrt(D))
NH = 4                 # heads per core
NC = 8                 # cores
NKC = HID // 128       # contraction chunks (128 wide)
SB = 512               # attention si-block / moving dim
NSB = S // SB          # si blocks
SH = S // 2            # s-half
F32 = mybir.dt.float32

MM_MODE = os.environ.get("BASS_MM_MODE", "fp32r")
MMDT = mybir.dt.float32r if MM_MODE == "fp32r" else mybir.dt.float32

LAST_RESULT = None  # BassKernelResults of the most recent run (for test harness)


def _build_nc():
    nc = bacc.Bacc("TRN2", target_bir_lowering=False, debug=False, num_devices=NC)

    xT = nc.dram_tensor("xT", [HID, S], F32, kind="ExternalInput")
    wqk = nc.dram_tensor("wqk", [2 * NH, 128, HID], F32, kind="ExternalInput")
    wv = nc.dram_tensor("wv", [NKC, 128, NH * 128], F32, kind="ExternalInput")
    woT = nc.dram_tensor("woT", [NH * 128, HID], F32, kind="ExternalInput")
    cosT = nc.dram_tensor("cosT", [D, S], F32, kind="ExternalInput")
    sinST = nc.dram_tensor("sinST", [D, S], F32, kind="ExternalInput")
    maskadd = nc.dram_tensor("maskadd", [128, 128], F32, kind="ExternalInput")
    out = nc.dram_tensor("out", [S, HID], F32, kind="ExternalOutput")

    with tile.TileContext(nc) as tc:
        with tc.tile_pool(name="pconst", bufs=1) as pconst, \
             tc.tile_pool(name="pqk", bufs=1) as pqk, \
             tc.tile_pool(name="pvn", bufs=1) as pvn:

            ones_f = pconst.tile([128, 4], F32, name="ones_f")
            nc.vector.memset(ones_f, 1.0)
            ones4 = pconst.tile([128, 4], MMDT, name="ones4")
            nc.vector.tensor_copy(ones4, ones_f)
            tri_t = pconst.tile([128, 128], F32, name="tri")
            nc.scalar.dma_start(out=tri_t, in_=maskadd[:, :])

            qT = [pqk.tile([128, S], MMDT, name=f"qT_{h}") for h in range(NH)]
            kT = [pqk.tile([128, S], MMDT, name=f"kT_{h}") for h in range(NH)]
            vn = [pvn.tile([128, 4, NH * 128], MMDT, name=f"vn_{g}")
                  for g in range(4)]
            outT = qT  # attention output aliases qT per si-block

            # ---- Phase A: q/k/v projection + RoPE, per s-half ----
            with tc.tile_pool(name="px", bufs=1) as px, \
                 tc.tile_pool(name="pwq", bufs=3) as pwq, \
                 tc.tile_pool(name="pwv", bufs=4) as pwvp, \
                 tc.tile_pool(name="ptrig", bufs=1) as ptrig, \
                 tc.tile_pool(name="psh", bufs=1) as psh:
                for half in range(2):
                    s0 = half * SH
                    xh = []
                    for kc in range(NKC):
                        xt = px.tile([128, SH], MMDT, name=f"xh{kc}")
                        src = xT[kc * 128:(kc + 1) * 128, s0:s0 + SH]
                        if half == 0 and kc < 4:
                            # single-DMA completion BW is ~40 GB/s: split the
                            # startup-critical chunks across two rings
                            nc.sync.dma_start(
                                out=xt[:, 0:SB],
                                in_=src[:, 0:SB].bitcast(MMDT))
                            nc.gpsimd.dma_start(
                                out=xt[:, SB:SH],
                                in_=src[:, SB:SH].bitcast(MMDT))
                        else:
                            eng = nc.sync if kc % 2 == 0 else nc.gpsimd
                            eng.dma_start(out=xt, in_=src.bitcast(MMDT))
                        xh.append(xt)
                    cos_t = ptrig.tile([D, SH], F32, name="cosT")
                    sin_t = ptrig.tile([D, SH], F32, name="sinST")
                    nc.scalar.dma_start(out=cos_t, in_=cosT[:, s0:s0 + SH])
                    nc.scalar.dma_start(out=sin_t, in_=sinST[:, s0:s0 + SH])

                    # q/k projection, transposed output [d, s], then RoPE.
                    # All PSUM->SBUF copies on ACT: the DVE FIFO holds only
                    # RoPE ops, so a late cos/sin or shuffle never delays
                    # PSUM bank recycling.
                    ppa_cm = tc.tile_pool(name="ppa", bufs=8, space="PSUM")
                    ppa = ppa_cm.__enter__()
                    for h in range(NH):
                        for kind, dst in ((0, qT[h]), (1, kT[h])):
                            ot = kind * NH + h
                            wt = pwq.tile([128, HID], MMDT, name="wqk")
                            for pc in range(2):
                                nc.scalar.dma_start(
                                    out=wt[:, pc * 1024:(pc + 1) * 1024],
                                    in_=wqk[ot][:, pc * 1024:(pc + 1) * 1024]
                                    .bitcast(MMDT))
                            ps0 = ppa.tile([128, SB], F32, name="qkps")
                            ps1 = ppa.tile([128, SB], F32, name="qkps")
                            for kc in range(NKC):
                                nc.tensor.matmul(
                                    ps0, wt[:, kc * 128:(kc + 1) * 128],
                                    xh[kc][:, 0:SB],
                                    start=(kc == 0), stop=(kc == NKC - 1))
                                nc.tensor.matmul(
                                    ps1, wt[:, kc * 128:(kc + 1) * 128],
                                    xh[kc][:, SB:SH],
                                    start=(kc == 0), stop=(kc == NKC - 1))
                            nc.scalar.copy(out=dst[:, s0:s0 + SB], in_=ps0)
                            nc.scalar.copy(out=dst[:, s0 + SB:s0 + SH], in_=ps1)
                            # RoPE in place (rotate-half partition swap by DMA)
                            sl = dst[:, s0:s0 + SH]
                            sh_t = psh.tile([128, SH], MMDT, name="shuf")
                            nc.gpsimd.dma_start(out=sh_t[0:64, :],
                                                in_=dst[64:128, s0:s0 + SH])
                            nc.gpsimd.dma_start(out=sh_t[64:128, :],
                                                in_=dst[0:64, s0:s0 + SH])
                            nc.vector.tensor_mul(sh_t, sh_t, sin_t)
                            nc.vector.tensor_mul(sl, sl, cos_t)
                            nc.vector.tensor_add(sl, sl, sh_t)
                    ppa_cm.__exit__(None, None, None)

                    # v projection, natural layout [s, 4 heads x d]: one kc
                    # pass over 8 PSUM banks, so each x chunk's slot retires
                    # at its kc step and the next half's refill overlaps the
                    # whole pass
                    pvp_cm = tc.tile_pool(name="pvp", bufs=8, space="PSUM")
                    pvp = pvp_cm.__enter__()
                    wv_t = []
                    for kc in range(NKC):
                        wvt = pwvp.tile([128, NH * 128], MMDT, name="wv")
                        nc.scalar.dma_start(out=wvt, in_=wv[kc].bitcast(MMDT))
                        wv_t.append(wvt)
                    vps = [pvp.tile([128, NH * 128], F32, name="vps")
                           for _ in range(8)]
                    for kc in range(NKC):
                        for st in range(8):
                            nc.tensor.matmul(
                                vps[st], xh[kc][:, st * 128:(st + 1) * 128],
                                wv_t[kc],
                                start=(kc == 0), stop=(kc == NKC - 1))
                    for st in range(8):
                        sg = half * 8 + st   # global s-chunk
                        nc.scalar.copy(out=vn[sg // 4][:, sg % 4, :],
                                       in_=vps[st])
                    pvp_cm.__exit__(None, None, None)

            # o-proj weights: issued at B start, resident long before C
            pwo_cm = tc.tile_pool(name="pwo", bufs=1)
            pwo = pwo_cm.__enter__()
            wo_t = []
            for h in range(NH):
                wot = pwo.tile([128, HID], MMDT, name=f"wo{h}")
                nc.scalar.dma_start(
                    out=wot,
                    in_=woT[h * 128:(h + 1) * 128, :].bitcast(MMDT))
                wo_t.append(wot)

            # ---- Phase B: attention per (si-block round, head) ----
            # Softmax denominator: e-tiles are accumulated on DVE/GpSimd into
            # e_acc (instead of streaming every chunk through a [128,4]-ones
            # matmul -> removes ~70K columns from the tensor stream); one
            # 512-col ones-matmul per (h, sib) reduces e_acc partitions, then
            # DVE reciprocal_approx_fast (18 bits) replaces the ACT ln/exp
            # pair. The 4 stages (ones-mm -> rec -> broadcast -> normalize
            # mul) advance one step per chunk-pair slot so every op reaches
            # its in-order FIFO only when its inputs are already done.
            pending = []   # [stage, h, si0, o_ps, (pair, lo, start), l4, rb]

            with tc.tile_pool(name="pexp", bufs=4) as pexp, \
                 tc.tile_pool(name="ppr", bufs=4) as ppr, \
                 tc.tile_pool(name="prr", bufs=2) as prr, \
                 tc.tile_pool(name="prb", bufs=2) as prb, \
                 tc.tile_pool(name="psc", bufs=3, space="PSUM") as psc, \
                 tc.tile_pool(name="plp", bufs=1, space="PSUM") as plp, \
                 tc.tile_pool(name="pop", bufs=3, space="PSUM") as pop, \
                 tc.tile_pool(name="pft", bufs=1, space="PSUM") as pft, \
                 tc.tile_pool(name="pst", bufs=4) as pst:

                # phase-C units (partial o-projection), interleaved into B
                # with a one-round lag so their inputs (normalized outT) are
                # long done and they fill B's dependency-stall gaps
                cu = 0

                def emit_c_st(st):
                    nonlocal cu
                    for ob in range(HID // SB):
                        fin = pft.tile([128, SB], F32, name="fin")
                        for hh in range(NH):
                            nc.tensor.matmul(
                                fin, outT[hh][:, st * 128:(st + 1) * 128],
                                wo_t[hh][:, ob * SB:(ob + 1) * SB],
                                start=(hh == 0), stop=(hh == NH - 1))
                        stg = pst.tile([128, SB], F32, name="stg")
                        if cu % 2 == 0:
                            nc.scalar.copy(out=stg, in_=fin)
                        else:
                            nc.vector.tensor_copy(stg, fin)
                        eng = nc.sync if cu % 2 == 0 else nc.gpsimd
                        cu += 1
                        eng.dma_start(
                            out=out[st * 128:(st + 1) * 128,
                                    ob * SB:(ob + 1) * SB],
                            in_=stg)

                def advance(item):
                    st = item[0]
                    _, h, si0, o_ps, last_pair, l4, rb = item
                    if st == 0:
                        # final ones-matmul of this head's denominator
                        pr, lo, first = last_pair
                        nc.tensor.matmul(l4[:, lo:], ones4, pr[:, lo:],
                                         start=first, stop=True)
                    elif st == 1:
                        rec = prr.tile([1, SB], F32, name="rec")
                        nc.vector.reciprocal_approx_fast(
                            out=rec, in_=l4[0:1, :])
                        rb = prb.tile([128, SB], F32, name="rb")
                        nc.gpsimd.partition_broadcast(rb, rec)
                        item[6] = rb
                    else:
                        nc.vector.tensor_mul(outT[h][:, si0:si0 + SB],
                                             o_ps, rb)
                    item[0] += 1
                    return item[0] >= 3

                def slot(budget):
                    done = 0
                    for item in list(pending):
                        if done >= budget:
                            break
                        if advance(item):
                            pending.remove(item)
                        done += 1

                for sib in range(NSB):
                    si0 = sib * SB
                    nch = 4 * (sib + 1)
                    for h in range(NH):
                        o_ps = pop.tile([128, SB], F32, name="ops")
                        l4 = plp.tile([4, SB], F32, name="l4")
                        prev_pair = None
                        for cp in range(nch // 2):
                            e_t = pexp.tile([128, 2, SB], MMDT, name="exp")
                            los = []
                            for j in range(2):
                                cj = cp * 2 + j
                                dg = cj - (nch - 4)
                                lo = dg * 128 if dg > 0 else 0
                                s_ps = psc.tile([128, SB], F32, name="sps")
                                los.append((cj, lo, s_ps))
                                nc.tensor.matmul(
                                    s_ps[:, lo:],
                                    kT[h][:, cj * 128:(cj + 1) * 128],
                                    qT[h][:, si0 + lo:si0 + SB],
                                    start=True, stop=True)
                                if dg >= 0:
                                    nc.vector.tensor_add(
                                        s_ps[:, lo:lo + 128],
                                        s_ps[:, lo:lo + 128], tri_t)
                            # exp only over the valid [lo:] ranges (the stale
                            # sub-diagonal region was ~15% wasted ACT time)
                            for j in range(2):
                                _, lo, s_ps = los[j]
                                nc.scalar.activation(
                                    out=e_t[:, j, lo:],
                                    in_=s_ps[:, lo:],
                                    func=mybir.ActivationFunctionType.Exp,
                                    scale=SCALE)
                            # previous heads' deferred normalize stages
                            slot(2 if sib == 0 else 1)
                            # independent pair-sum of the two e tiles. DVE
                            # ONLY: gpsimd tensor ops swap in a DSP library
                            # (MODIFY_POOL_CONFIG LOAD_LIB ~6us) evicting the
                            # partition_broadcast lib -> measured disaster.
                            lo0, lo1 = los[0][1], los[1][1]
                            pr = ppr.tile([128, SB], MMDT, name="pair")
                            if lo0 == lo1:
                                nc.vector.tensor_add(pr[:, lo0:],
                                                     e_t[:, 0, lo0:],
                                                     e_t[:, 1, lo1:])
                            else:
                                nc.vector.tensor_copy(pr[:, lo0:lo1],
                                                      e_t[:, 0, lo0:lo1])
                                nc.vector.tensor_add(pr[:, lo1:],
                                                     e_t[:, 0, lo1:],
                                                     e_t[:, 1, lo1:])
                            # ones-matmul for the PREVIOUS pair (one cp of
                            # slack so the tensor FIFO never waits on it)
                            if prev_pair is not None:
                                ppr_t, plo, first = prev_pair
                                nc.tensor.matmul(
                                    l4[:, plo:], ones4, ppr_t[:, plo:],
                                    start=first, stop=False)
                            prev_pair = (pr, lo0, cp == 0)
                            for j in range(2):
                                cj, lo, _ = los[j]
                                nc.tensor.matmul(
                                    o_ps[:, lo:],
                                    vn[cj // 4][:, cj % 4,
                                                h * 128:(h + 1) * 128],
                                    e_t[:, j, lo:],
                                    start=(cj == 0), stop=(cj == nch - 1))
                        pending.append([0, h, si0, o_ps, prev_pair, l4, None])
                        # one C st-group of the previous round per head: its
                        # inputs were normalized a full round ago, so these
                        # dense matmuls fill B's dependency-stall gaps
                        if sib >= 1:
                            emit_c_st(4 * (sib - 1) + h)
                    # round end: drain so at most one head stays in flight
                    while len(pending) > 1:
                        if advance(pending[0]):
                            pending.pop(0)
                while pending:
                    if advance(pending[0]):
                        pending.pop(0)
                # C tail: the last round's st-groups
                for st in range(4 * (NSB - 1), S // 128):
                    emit_c_st(st)

            pwo_cm.__exit__(None, None, None)

    # Force exp and ln onto the single `natural_log_exp_and_others` ACT
    # table set: with the default map the table-load pass alternates between
    # the exp-only and ln-only sets (~2.7us per reload on ScalarE). Blank
    # the single-function sets (positions preserved, so set ids stay valid)
    # so both functions resolve to the combined set -> one load.
    import concourse.bacc as _bacc_mod
    import concourse.hw_specs as _hw_specs
    _orig_tables = _hw_specs.get_activation_tables

    def _patched_tables(arch):
        t = dict(_orig_tables(arch))
        for name in ("exp_and_others", "exp_and_friends", "natural_log"):
            if name in t:
                t[name] = set()
        return t

    _bacc_mod.get_activation_tables = _patched_tables
    try:
        nc.compile()
    finally:
        _bacc_mod.get_activation_tables = _orig_tables
    return nc


_NC_CACHE = None


def _get_nc():
    global _NC_CACHE
    if _NC_CACHE is None:
        _NC_CACHE = _build_nc()
    return _NC_CACHE


def _host_inputs(x, w_qkv, w_o):
    """Per-core input maps (sharding + contiguous-DMA layout prep on host)."""
    inv_freq = 1.0 / (THETA ** (np.arange(0, D, 2, dtype=np.float64) / D))
    pos = np.arange(S, dtype=np.float64)
    freqs = pos[:, None] * inv_freq[None, :]          # (S, D/2)
    emb = np.concatenate([freqs, freqs], axis=-1)     # (S, D)
    cosT = np.ascontiguousarray(np.cos(emb).T.astype(np.float32))   # (D, S)
    sign = np.concatenate([-np.ones(D // 2), np.ones(D // 2)])
    sinST = np.ascontiguousarray((sign[None, :] * np.sin(emb)).T
                                 .astype(np.float32))               # (D, S)
    # additive causal triangle for a diagonal 128x128 block of scores^T:
    # keep (add 0) when sj_local <= si_local, else -1e30
    p = np.arange(128)[:, None]
    f = np.arange(128)[None, :]
    maskadd = np.where(p <= f, 0.0, -1e30).astype(np.float32)       # (128, 128)

    xTb = [np.ascontiguousarray(x[b].T) for b in range(B)]          # (HID, S)
    in_maps = []
    for c in range(NC):
        b, g = c // 4, c % 4
        rows = slice(g * NH * D, (g + 1) * NH * D)
        wq = w_qkv[0 * HID:1 * HID][rows]             # (512, 2048)
        wk = w_qkv[1 * HID:2 * HID][rows]
        wvm = w_qkv[2 * HID:3 * HID][rows]
        # wqk[ot][p, kc*128+od] = w[ot*128+od, kc*128+p]
        wqk_arr = np.empty((2 * NH, 128, HID), dtype=np.float32)
        for kind, wm in ((0, wq), (1, wk)):
            for h in range(NH):
                wT = wm[h * 128:(h + 1) * 128].T      # (2048 hid, 128 od)
                wqk_arr[kind * NH + h] = (
                    wT.reshape(NKC, 128, 128).transpose(1, 0, 2)
                    .reshape(128, HID))
        # wv[kc][p, j] = w_v[j, kc*128+p]
        wv_arr = np.ascontiguousarray(
            wvm.T.reshape(NKC, 128, NH * 128)).astype(np.float32)
        woT = np.ascontiguousarray(w_o[:, rows].T).astype(np.float32)
        in_maps.append({
            "xT": xTb[b], "wqk": wqk_arr, "wv": wv_arr, "woT": woT,
            "cosT": cosT, "sinST": sinST, "maskadd": maskadd,
        })
    return in_maps


def kernel(x, w_qkv, w_o):
    global LAST_RESULT
    x = np.asarray(x, dtype=np.float32)
    w_qkv = np.asarray(w_qkv, dtype=np.float32)
    w_o = np.asarray(w_o, dtype=np.float32)

    nc = _get_nc()
    in_maps = _host_inputs(x, w_qkv, w_o)
    trace = bool(int(os.environ.get("BASS_KERNEL_TRACE", "0")))
    last_exc = None
    for _attempt in range(3):
        try:
            res = run_bass_kernel_spmd(
                nc, in_maps, core_ids=list(range(NC)),
                trace=trace, trace_cores=list(range(NC)) if trace else None)
            break
        except Exception as e:  # transient NRT device errors: retry
            last_exc = e
    else:
        raise last_exc
    LAST_RESULT = res

    out = np.empty((B, S, HID), dtype=np.float32)
    for b in range(B):
        acc = np.zeros((S, HID), dtype=np.float64)
        for g in range(4):
            acc += res.results[b * 4 + g]["out"]
        out[b] = acc.astype(np.float32)
    return out



# revision 19
# speedup vs baseline: 1.6283x; 1.0889x over previous
"""Causal self-attention with RoPE on 8 Trainium2 NeuronCores (v5).

Problem: B=2, S=2048, H=16 heads, D=128, HID=2048, fp32.
  qkv = x @ w_qkv.T ; RoPE(q, k) ; causal softmax(q k^T / sqrt(D)) @ v ; out @ w_o.T

Sharding (hardcoded): core c handles batch b = c // 4 and head group
g = c % 4 (heads 4g..4g+4). Each core computes a partial (S, HID) output
contracted over its 512 hidden dims of the o-projection; the host sums the 4
partials per batch.

All matmuls run in fp32r (TF32-class): at moving dims >=256 fp32r streams
1 column/cycle at full clock; bf16 draws more PE power and downclocks
(~259 ns vs ~227 ns per 512-col matmul, measured).

Engine discipline (engine queues are strict in-order FIFOs, so latency
coupling matters more than busy%):
 - ACT: all PSUM->SBUF copies + exp/ln. DVE: RoPE + mask adds + normalize
   muls only, so a copy never queues behind a stalled RoPE/normalize.
 - The softmax reciprocal is split in three (ln+exp on ACT / broadcast on
   GpSimd / multiply on DVE) and each stage is emitted only at a program
   point where its inputs are already done, so no FIFO head-blocks.
 - Phase C is NOT interleaved into B: C units gate on normalize chains and
   would stall the in-order PE stream (measured regression).

DMA discipline: per-dma_start completion bandwidth is only ~40 GB/s
(aggregate across queues ~330 GB/s), so the first x chunks are split into
column-halves across two rings, weights are host-prepped contiguous, and
issue order matches consumption order. The v projection runs as a single
kc pass so each x chunk's SBUF slot retires early — the next half's x
refills stream in under the whole v pass and the half boundary has no
DMA bubble.
"""

import os

import numpy as np

import concourse.bacc as bacc
import concourse.tile as tile
from concourse import mybir
from concourse.bass_utils import run_bass_kernel_spmd

B, S, H, D = 2, 2048, 16, 128
HID = H * D
THETA = 10000.0
SCALE = 1.0 / float(np.sqrt(D))
NH = 4                 # heads per core
NC = 8                 # cores
NKC = HID // 128       # contraction chunks (128 wide)
SB = 512               # attention si-block / moving dim
NSB = S // SB          # si blocks
SH = S // 2            # s-half
F32 = mybir.dt.float32

MM_MODE = os.environ.get("BASS_MM_MODE", "fp32r")
MMDT = mybir.dt.float32r if MM_MODE == "fp32r" else mybir.dt.float32

LAST_RESULT = None  # BassKernelResults of the most recent run (for test harness)


def _build_nc():
    nc = bacc.Bacc("TRN2", target_bir_lowering=False, debug=False, num_devices=NC)

    xT = nc.dram_tensor("xT", [HID, S], F32, kind="ExternalInput")
    wqkg = nc.dram_tensor("wqkg", [2 * NKC, 128, 4 * 128], F32,
                          kind="ExternalInput")
    wv = nc.dram_tensor("wv", [NKC, 128, NH * 128], F32, kind="ExternalInput")
    woT = nc.dram_tensor("woT", [NH * 128, HID], F32, kind="ExternalInput")
    cosT = nc.dram_tensor("cosT", [D, S], F32, kind="ExternalInput")
    sinST = nc.dram_tensor("sinST", [D, S], F32, kind="ExternalInput")
    maskadd = nc.dram_tensor("maskadd", [128, 128], F32, kind="ExternalInput")
    out = nc.dram_tensor("out", [S, HID], F32, kind="ExternalOutput")

    with tile.TileContext(nc) as tc:
        with tc.tile_pool(name="pconst", bufs=1) as pconst, \
             tc.tile_pool(name="pqk", bufs=1) as pqk, \
             tc.tile_pool(name="pvn", bufs=1) as pvn:

            ones_f = pconst.tile([128, 4], F32, name="ones_f")
            nc.vector.memset(ones_f, 1.0)
            ones4 = pconst.tile([128, 4], MMDT, name="ones4")
            nc.vector.tensor_copy(ones4, ones_f)
            tri_t = pconst.tile([128, 128], F32, name="tri")
            nc.scalar.dma_start(out=tri_t, in_=maskadd[:, :])

            qT = [pqk.tile([128, S], MMDT, name=f"qT_{h}") for h in range(NH)]
            kT = [pqk.tile([128, S], MMDT, name=f"kT_{h}") for h in range(NH)]
            vn = [pvn.tile([128, 4, NH * 128], MMDT, name=f"vn_{g}")
                  for g in range(4)]
            outT = qT  # attention output aliases qT per si-block

            # ---- Phase A: q/k/v projection + RoPE, kc-major groups ----
            # Weights stream as [128,512] chunks on scalar in consumption
            # order; x chunks split col-wise over sync+gpsimd. The first
            # matmul starts once w-chunk 0 + x-chunk 0a land (~6us, vs ~24us
            # for the old full-tile weight gate), and group 0's kc loop is
            # paced by arrival (~2.2us/kc) instead of stalling in bursts.
            with tc.tile_pool(name="px", bufs=1) as px, \
                 tc.tile_pool(name="pwg", bufs=6) as pwg, \
                 tc.tile_pool(name="pwv", bufs=8) as pwvp, \
                 tc.tile_pool(name="ptrig", bufs=1) as ptrig, \
                 tc.tile_pool(name="psh", bufs=1) as psh:
                xh = [px.tile([128, SH], MMDT, name=f"xh{kc}")
                      for kc in range(NKC)]

                def emit_x(half, kc):
                    s0h = half * SH
                    src = xT[kc * 128:(kc + 1) * 128, s0h:s0h + SH]
                    nc.sync.dma_start(out=xh[kc][:, 0:SB],
                                      in_=src[:, 0:SB].bitcast(MMDT))
                    nc.gpsimd.dma_start(out=xh[kc][:, SB:SH],
                                        in_=src[:, SB:SH].bitcast(MMDT))

                trig = []

                def emit_trig(half):
                    s0h = half * SH
                    cos_t = ptrig.tile([D, SH], F32, name="cosT")
                    sin_t = ptrig.tile([D, SH], F32, name="sinST")
                    nc.sync.dma_start(out=cos_t, in_=cosT[:, s0h:s0h + SH])
                    nc.gpsimd.dma_start(out=sin_t, in_=sinST[:, s0h:s0h + SH])
                    trig.append((cos_t, sin_t))

                for kc in range(6):
                    emit_x(0, kc)
                emit_trig(0)
                for kc in range(6, NKC):
                    emit_x(0, kc)

                for half in range(2):
                    s0 = half * SH
                    cos_t, sin_t = trig[half]
                    ppa_cm = tc.tile_pool(name="ppa", bufs=1, space="PSUM")
                    ppa = ppa_cm.__enter__()
                    ps = [ppa.tile([128, SB], F32, name=f"qkps{i}")
                          for i in range(8)]
                    for g in range(2):
                        wg_t = []
                        for kc in range(NKC):
                            wt = pwg.tile([128, 4 * 128], MMDT, name="wg")
                            nc.scalar.dma_start(
                                out=wt,
                                in_=wqkg[g * NKC + kc].bitcast(MMDT))
                            wg_t.append(wt)
                        for kc in range(NKC):
                            for j in range(2):
                                for o4 in range(4):
                                    nc.tensor.matmul(
                                        ps[o4 * 2 + j],
                                        wg_t[kc][:, o4 * 128:(o4 + 1) * 128],
                                        xh[kc][:, j * SB:(j + 1) * SB],
                                        start=(kc == 0), stop=(kc == NKC - 1))
                        # copies on ACT, RoPE on DVE; rotate-half partition
                        # swap DMAs split over sync+gpsimd so neither queue
                        # carries the whole 8MB/half (the A1 tail of these
                        # was stalling B's first mask-adds and broadcast)
                        for o4 in range(4):
                            kind, hh = o4 % 2, g * 2 + o4 // 2
                            dst = (qT if kind == 0 else kT)[hh]
                            nc.scalar.copy(out=dst[:, s0:s0 + SB],
                                           in_=ps[o4 * 2 + 0])
                            nc.scalar.copy(out=dst[:, s0 + SB:s0 + SH],
                                           in_=ps[o4 * 2 + 1])
                            sl = dst[:, s0:s0 + SH]
                            sh_t = psh.tile([128, SH], MMDT, name="shuf")
                            for c0, c1, eng in ((0, SB, nc.sync),
                                                (SB, SH, nc.gpsimd)):
                                eng.dma_start(
                                    out=sh_t[0:64, c0:c1],
                                    in_=dst[64:128, s0 + c0:s0 + c1])
                                eng.dma_start(
                                    out=sh_t[64:128, c0:c1],
                                    in_=dst[0:64, s0 + c0:s0 + c1])
                            nc.vector.tensor_mul(sh_t, sh_t, sin_t)
                            nc.vector.tensor_mul(sl, sl, cos_t)
                            nc.vector.tensor_add(sl, sl, sh_t)
                    ppa_cm.__exit__(None, None, None)

                    # v projection, natural layout [s, 4 heads x d]: one kc
                    # pass over 8 PSUM banks; each x chunk's last read is its
                    # kc step, so the half-1 refill for that chunk is emitted
                    # right behind it (just-in-time, no queue head-blocking)
                    pvp_cm = tc.tile_pool(name="pvp", bufs=8, space="PSUM")
                    pvp = pvp_cm.__enter__()
                    wv_t = []
                    for kc in range(NKC):
                        wvt = pwvp.tile([128, NH * 128], MMDT, name="wv")
                        nc.sync.dma_start(out=wvt, in_=wv[kc].bitcast(MMDT))
                        wv_t.append(wvt)
                    vps = [pvp.tile([128, NH * 128], F32, name="vps")
                           for _ in range(8)]
                    for kc in range(NKC):
                        for st in range(8):
                            nc.tensor.matmul(
                                vps[st], xh[kc][:, st * 128:(st + 1) * 128],
                                wv_t[kc],
                                start=(kc == 0), stop=(kc == NKC - 1))
                        if half == 0:
                            emit_x(1, kc)
                            if kc == 5:
                                emit_trig(1)
                    for st in range(8):
                        sg = half * 8 + st   # global s-chunk
                        nc.scalar.copy(out=vn[sg // 4][:, sg % 4, :],
                                       in_=vps[st])
                    pvp_cm.__exit__(None, None, None)

            # o-proj weights: issued at B start, resident long before C
            pwo_cm = tc.tile_pool(name="pwo", bufs=1)
            pwo = pwo_cm.__enter__()
            wo_t = []
            for h in range(NH):
                wot = pwo.tile([128, HID], MMDT, name=f"wo{h}")
                nc.scalar.dma_start(
                    out=wot,
                    in_=woT[h * 128:(h + 1) * 128, :].bitcast(MMDT))
                wo_t.append(wot)

            # ---- Phase B: attention per (si-block round, head) ----
            # Softmax denominator: e-tiles are accumulated on DVE/GpSimd into
            # e_acc (instead of streaming every chunk through a [128,4]-ones
            # matmul -> removes ~70K columns from the tensor stream); one
            # 512-col ones-matmul per (h, sib) reduces e_acc partitions, then
            # DVE reciprocal_approx_fast (18 bits) replaces the ACT ln/exp
            # pair. The 4 stages (ones-mm -> rec -> broadcast -> normalize
            # mul) advance one step per chunk-pair slot so every op reaches
            # its in-order FIFO only when its inputs are already done.
            pending = []   # [stage, h, si0, o_ps, (pair, lo, start), l4, rb]

            with tc.tile_pool(name="pexp", bufs=4) as pexp, \
                 tc.tile_pool(name="ppr", bufs=4) as ppr, \
                 tc.tile_pool(name="prr", bufs=2) as prr, \
                 tc.tile_pool(name="prb", bufs=2) as prb, \
                 tc.tile_pool(name="psc", bufs=3, space="PSUM") as psc, \
                 tc.tile_pool(name="plp", bufs=1, space="PSUM") as plp, \
                 tc.tile_pool(name="pop", bufs=3, space="PSUM") as pop, \
                 tc.tile_pool(name="pft", bufs=1, space="PSUM") as pft, \
                 tc.tile_pool(name="pst", bufs=4) as pst:

                # phase-C units (partial o-projection), interleaved into B
                # with a one-round lag so their inputs (normalized outT) are
                # long done and they fill B's dependency-stall gaps
                cu = 0

                def emit_c_st(st):
                    nonlocal cu
                    for ob in range(HID // SB):
                        fin = pft.tile([128, SB], F32, name="fin")
                        for hh in range(NH):
                            nc.tensor.matmul(
                                fin, outT[hh][:, st * 128:(st + 1) * 128],
                                wo_t[hh][:, ob * SB:(ob + 1) * SB],
                                start=(hh == 0), stop=(hh == NH - 1))
                        stg = pst.tile([128, SB], F32, name="stg")
                        if cu % 2 == 0:
                            nc.scalar.copy(out=stg, in_=fin)
                        else:
                            nc.vector.tensor_copy(stg, fin)
                        eng = nc.sync if cu % 2 == 0 else nc.gpsimd
                        cu += 1
                        eng.dma_start(
                            out=out[st * 128:(st + 1) * 128,
                                    ob * SB:(ob + 1) * SB],
                            in_=stg)

                def advance(item):
                    st = item[0]
                    _, h, si0, o_ps, last_pair, l4, rb = item
                    if st == 0:
                        # final ones-matmul of this head's denominator
                        pr, lo, first = last_pair
                        nc.tensor.matmul(l4[:, lo:], ones4, pr[:, lo:],
                                         start=first, stop=True)
                    elif st == 1:
                        rec = prr.tile([1, SB], F32, name="rec")
                        nc.vector.reciprocal_approx_fast(
                            out=rec, in_=l4[0:1, :])
                        rb = prb.tile([128, SB], F32, name="rb")
                        nc.gpsimd.partition_broadcast(rb, rec)
                        item[6] = rb
                    else:
                        nc.vector.tensor_mul(outT[h][:, si0:si0 + SB],
                                             o_ps, rb)
                    item[0] += 1
                    return item[0] >= 3

                def slot(budget):
                    done = 0
                    for item in list(pending):
                        if done >= budget:
                            break
                        if advance(item):
                            pending.remove(item)
                        done += 1

                for sib in range(NSB):
                    si0 = sib * SB
                    nch = 4 * (sib + 1)
                    for h in range(NH):
                        o_ps = pop.tile([128, SB], F32, name="ops")
                        l4 = plp.tile([4, SB], F32, name="l4")
                        prev_pair = None
                        for cp in range(nch // 2):
                            e_t = pexp.tile([128, 2, SB], MMDT, name="exp")
                            los = []
                            for j in range(2):
                                cj = cp * 2 + j
                                dg = cj - (nch - 4)
                                lo = dg * 128 if dg > 0 else 0
                                s_ps = psc.tile([128, SB], F32, name="sps")
                                los.append((cj, lo, s_ps))
                                nc.tensor.matmul(
                                    s_ps[:, lo:],
                                    kT[h][:, cj * 128:(cj + 1) * 128],
                                    qT[h][:, si0 + lo:si0 + SB],
                                    start=True, stop=True)
                                if dg >= 0:
                                    nc.vector.tensor_add(
                                        s_ps[:, lo:lo + 128],
                                        s_ps[:, lo:lo + 128], tri_t)
                            # exp only over the valid [lo:] ranges (the stale
                            # sub-diagonal region was ~15% wasted ACT time)
                            for j in range(2):
                                _, lo, s_ps = los[j]
                                nc.scalar.activation(
                                    out=e_t[:, j, lo:],
                                    in_=s_ps[:, lo:],
                                    func=mybir.ActivationFunctionType.Exp,
                                    scale=SCALE)
                            # previous heads' deferred normalize stages
                            slot(2 if sib == 0 else 1)
                            # independent pair-sum of the two e tiles. DVE
                            # ONLY: gpsimd tensor ops swap in a DSP library
                            # (MODIFY_POOL_CONFIG LOAD_LIB ~6us) evicting the
                            # partition_broadcast lib -> measured disaster.
                            lo0, lo1 = los[0][1], los[1][1]
                            pr = ppr.tile([128, SB], MMDT, name="pair")
                            if lo0 == lo1:
                                nc.vector.tensor_add(pr[:, lo0:],
                                                     e_t[:, 0, lo0:],
                                                     e_t[:, 1, lo1:])
                            else:
                                nc.vector.tensor_copy(pr[:, lo0:lo1],
                                                      e_t[:, 0, lo0:lo1])
                                nc.vector.tensor_add(pr[:, lo1:],
                                                     e_t[:, 0, lo1:],
                                                     e_t[:, 1, lo1:])
                            # ones-matmul for the PREVIOUS pair (one cp of
                            # slack so the tensor FIFO never waits on it)
                            if prev_pair is not None:
                                ppr_t, plo, first = prev_pair
                                nc.tensor.matmul(
                                    l4[:, plo:], ones4, ppr_t[:, plo:],
                                    start=first, stop=False)
                            prev_pair = (pr, lo0, cp == 0)
                            for j in range(2):
                                cj, lo, _ = los[j]
                                nc.tensor.matmul(
                                    o_ps[:, lo:],
                                    vn[cj // 4][:, cj % 4,
                                                h * 128:(h + 1) * 128],
                                    e_t[:, j, lo:],
                                    start=(cj == 0), stop=(cj == nch - 1))
                        pending.append([0, h, si0, o_ps, prev_pair, l4, None])
                        # one C st-group of the previous round per head: its
                        # inputs were normalized a full round ago, so these
                        # dense matmuls fill B's dependency-stall gaps
                        if sib >= 1:
                            emit_c_st(4 * (sib - 1) + h)
                    # round end: drain so at most one head stays in flight
                    while len(pending) > 1:
                        if advance(pending[0]):
                            pending.pop(0)
                while pending:
                    if advance(pending[0]):
                        pending.pop(0)

            # C tail (last round's st-groups) in its own pool block: B's
            # PSUM banks are free here, so fin can triple-buffer instead of
            # serializing each unit behind the previous stg copy
            with tc.tile_pool(name="pft2", bufs=3, space="PSUM") as pft2, \
                 tc.tile_pool(name="pst2", bufs=4) as pst2:
                u = 0
                for st in range(4 * (NSB - 1), S // 128):
                    for ob in range(HID // SB):
                        fin = pft2.tile([128, SB], F32, name="fin")
                        for hh in range(NH):
                            nc.tensor.matmul(
                                fin, outT[hh][:, st * 128:(st + 1) * 128],
                                wo_t[hh][:, ob * SB:(ob + 1) * SB],
                                start=(hh == 0), stop=(hh == NH - 1))
                        stg = pst2.tile([128, SB], F32, name="stg")
                        if u % 2 == 0:
                            nc.scalar.copy(out=stg, in_=fin)
                        else:
                            nc.vector.tensor_copy(stg, fin)
                        eng = nc.sync if u % 2 == 0 else nc.gpsimd
                        u += 1
                        eng.dma_start(
                            out=out[st * 128:(st + 1) * 128,
                                    ob * SB:(ob + 1) * SB],
                            in_=stg)

            pwo_cm.__exit__(None, None, None)

    # Force exp and ln onto the single `natural_log_exp_and_others` ACT
    # table set: with the default map the table-load pass alternates between
    # the exp-only and ln-only sets (~2.7us per reload on ScalarE). Blank
    # the single-function sets (positions preserved, so set ids stay valid)
    # so both functions resolve to the combined set -> one load.
    import concourse.bacc as _bacc_mod
    import concourse.hw_specs as _hw_specs
    _orig_tables = _hw_specs.get_activation_tables

    def _patched_tables(arch):
        t = dict(_orig_tables(arch))
        for name in ("exp_and_others", "exp_and_friends", "natural_log"):
            if name in t:
                t[name] = set()
        return t

    _bacc_mod.get_activation_tables = _patched_tables
    try:
        nc.compile()
    finally:
        _bacc_mod.get_activation_tables = _orig_tables
    return nc


_NC_CACHE = None


def _get_nc():
    global _NC_CACHE
    if _NC_CACHE is None:
        _NC_CACHE = _build_nc()
    return _NC_CACHE


def _host_inputs(x, w_qkv, w_o):
    """Per-core input maps (sharding + contiguous-DMA layout prep on host)."""
    inv_freq = 1.0 / (THETA ** (np.arange(0, D, 2, dtype=np.float64) / D))
    pos = np.arange(S, dtype=np.float64)
    freqs = pos[:, None] * inv_freq[None, :]          # (S, D/2)
    emb = np.concatenate([freqs, freqs], axis=-1)     # (S, D)
    cosT = np.ascontiguousarray(np.cos(emb).T.astype(np.float32))   # (D, S)
    sign = np.concatenate([-np.ones(D // 2), np.ones(D // 2)])
    sinST = np.ascontiguousarray((sign[None, :] * np.sin(emb)).T
                                 .astype(np.float32))               # (D, S)
    # additive causal triangle for a diagonal 128x128 block of scores^T:
    # keep (add 0) when sj_local <= si_local, else -1e30
    p = np.arange(128)[:, None]
    f = np.arange(128)[None, :]
    maskadd = np.where(p <= f, 0.0, -1e30).astype(np.float32)       # (128, 128)

    xTb = [np.ascontiguousarray(x[b].T) for b in range(B)]          # (HID, S)
    in_maps = []
    for c in range(NC):
        b, g = c // 4, c % 4
        rows = slice(g * NH * D, (g + 1) * NH * D)
        wq = w_qkv[0 * HID:1 * HID][rows]             # (512, 2048)
        wk = w_qkv[1 * HID:2 * HID][rows]
        wvm = w_qkv[2 * HID:3 * HID][rows]
        # wqkg[g*NKC+kc][p][o4*128+od] = W_{kind,head}[od, kc*128+p], where
        # group g serves outputs o4 = (q,h=2g), (k,h=2g), (q,h=2g+1),
        # (k,h=2g+1) -- one [128,512] chunk per (group, kc) DMA
        wqkg_arr = np.empty((2 * NKC, 128, 4 * 128), dtype=np.float32)
        for g in range(2):
            for o4 in range(4):
                kind, hh = o4 % 2, g * 2 + o4 // 2
                wm = (wq if kind == 0 else wk)[hh * 128:(hh + 1) * 128]
                t = np.ascontiguousarray(wm.T).reshape(NKC, 128, 128)
                for kc in range(NKC):
                    wqkg_arr[g * NKC + kc][:, o4 * 128:(o4 + 1) * 128] = t[kc]
        # wv[kc][p, j] = w_v[j, kc*128+p]
        wv_arr = np.ascontiguousarray(
            wvm.T.reshape(NKC, 128, NH * 128)).astype(np.float32)
        woT = np.ascontiguousarray(w_o[:, rows].T).astype(np.float32)
        in_maps.append({
            "xT": xTb[b], "wqkg": wqkg_arr, "wv": wv_arr, "woT": woT,
            "cosT": cosT, "sinST": sinST, "maskadd": maskadd,
        })
    return in_maps


def kernel(x, w_qkv, w_o):
    global LAST_RESULT
    x = np.asarray(x, dtype=np.float32)
    w_qkv = np.asarray(w_qkv, dtype=np.float32)
    w_o = np.asarray(w_o, dtype=np.float32)

    nc = _get_nc()
    in_maps = _host_inputs(x, w_qkv, w_o)
    trace = bool(int(os.environ.get("BASS_KERNEL_TRACE", "0")))
    last_exc = None
    for _attempt in range(3):
        try:
            res = run_bass_kernel_spmd(
                nc, in_maps, core_ids=list(range(NC)),
                trace=trace, trace_cores=list(range(NC)) if trace else None)
            break
        except Exception as e:  # transient NRT device errors: retry
            last_exc = e
    else:
        raise last_exc
    LAST_RESULT = res

    out = np.empty((B, S, HID), dtype=np.float32)
    for b in range(B):
        acc = np.zeros((S, HID), dtype=np.float64)
        for g in range(4):
            acc += res.results[b * 4 + g]["out"]
        out[b] = acc.astype(np.float32)
    return out

